# revision 14
# baseline (speedup 1.0000x reference)
"""EdgeConv (PyG, aggr='max') Trainium2 kernel, 8-core SPMD.

Math: out_i = max_{e: dst(e)=i} relu(x_i @ W1.T + (x_src(e) - x_i) @ W2.T + b)
with W = [W1 | W2].  Rewriting:
    msg_e = relu(A_i + g_src(e)),  A = x @ (W1-W2).T + b,  g = x @ W2.T
Since A_i is constant within segment i and relu is monotone:
    out_i = relu(A_i + max_e g_src(e))
The reference's dst is repeat(arange(N), DEG), so segments are 16 consecutive
edges and the segment-max is a fixed-group reduce after routing each edge's
g row to its slot.

Two SPMD launches on 8 cores:

L1 (dense, node-parallel): host supplies the 6250-node shard pre-transposed,
   xT_aug [65, 6400] bf16, and wt [65, 64] = W2.T (+ zero ones-row).  One
   matmul per 512 nodes emits gT [64, 512] -> bulk store of gT [64, 6400].
   No PE transposes, all DMA transfers are bulk.

L2 (gather + segment max, edge-parallel): the g table is kept as 256B-stride
   pair rows gpair[r] = [g_{2r} | g_{2r+1}] (row 25000 = -3e38 sentinel), but
   each edge's descriptor transfers only the 128B half it needs: dma_gather's
   HBM address math is idx*stride_bytes_256*256 with the transfer size set
   independently by elem_size, so gathers with elem_size=64 bf16 from base
   +0B / +128B views fetch exactly g[src] for even/odd src with idx = src>>1
   (fits int16).  elem_size_bytes%256==0 is only a transpose-mode ucode
   restriction, so the instruction is emitted directly (the bass helper
   over-asserts).  Each node's 16 edges split unevenly between the two
   parity gathers, so the host sorts nodes by even-source count, tiles the
   sorted order into 128-node tiles, and gives each run of tiles uniform
   slot counts (K_even, K_odd) padded with sentinel indices; the segment max
   is then an in-place log2-halving elementwise max over the slot axis.
   A (+ bias, in grouped node order) is recomputed inside this launch on the
   otherwise-idle PE — per 128-node tile, matmul(lhsT=x_tile_aug, rhs=V1_aug)
   lands A directly node-major in PSUM, copied to SBUF on the idle ACT
   engine — then combine = relu(max(even,odd) + A).  All index/permutation
   prep is host-side; the host un-permutes the output rows at the end.
"""

import numpy as np

N_NODES = 50000
DEG = 16
C = 64
N_CORES = 8
NSH = N_NODES // N_CORES  # 6250 nodes per core
P = 128
NT = -(-NSH // P)  # 49 tiles of 128 sorted nodes
NPAD = NT * P  # 6272
NPAIR = N_NODES // 2 + 1  # pair rows + sentinel row
SENT_ROW = N_NODES // 2  # 25000
SENT = -3.0e38
KAUG = C + 1  # x channels + ones row
DN = 6400  # dense-phase padded node count (50 tiles)
MM_CH = 512  # matmul chunk (one PSUM bank)
MAXSL = 64  # max per-partition slots per gather instruction
MAXB = 16  # bucket budget after merging

_cache = {}


def _bf16():
    import ml_dtypes

    return ml_dtypes.bfloat16


def _raw_gather(nc, out_ap, in_ap, idxs_ap, num_idxs):
    """dma_gather with elem_size=64 bf16 (128B payload) on a 256B-stride
    table: bass.dma_gather asserts elem_size_bytes%256==0, but the ucode
    only needs that for transpose mode; emit the instruction directly."""
    import concourse.mybir as mybir

    g = nc.gpsimd
    elem_step = in_ap.ap[0][0]
    stride_bytes = elem_step * mybir.dt.size(in_ap.dtype)
    assert stride_bytes % 256 == 0 and stride_bytes // 256 < 256
    elem_size = in_ap.ap[-1][1]
    assert out_ap.ap[-1][1] == elem_size
    assert out_ap.ap[0][1] * out_ap.ap[1][1] == num_idxs and num_idxs % 128 == 0
    return g.add_instruction(
        mybir.InstDMAGatherAnt(
            name=g.bass.get_next_instruction_name(),
            ins=[
                *g.lower_ap_dma(in_ap, for_custom_bir_dma=True),
                g.lower_ap(idxs_ap),
                g.lower_val_access(g.to_reg(num_idxs)),
            ],
            outs=[g.lower_ap(out_ap)],
            transpose=False,
            num_idxs=num_idxs,
            elem_size=elem_size,
            stride_bytes_256=stride_bytes // 256,
            gen_mode=0,
            single_packet=False,
            queue_num=0,
            sbuf_tokens_per_rank=0,
            sbuf_free_dim_per_rank=0,
            sbuf_free_dim_pad_per_rank=0,
            sbuf_byte_offset=0,
        )
    )


def _build_dense():
    import concourse.bacc as bacc
    import concourse.mybir as mybir
    from concourse.tile import TileContext

    nc = bacc.Bacc("TRN2", target_bir_lowering=False, debug=False)
    bf16 = mybir.dt.bfloat16
    xw = nc.dram_tensor("xw", [KAUG, DN], bf16, kind="ExternalInput")
    wt = nc.dram_tensor("wt", [KAUG, C], bf16, kind="ExternalInput")
    ga = nc.dram_tensor("ga", [C, DN], bf16, kind="ExternalOutput")

    with TileContext(nc) as tc:
        with (
            tc.tile_pool(name="const", bufs=1) as cpool,
            tc.tile_pool(name="sbuf", bufs=1) as pool,
            tc.tile_pool(name="psum", bufs=4, space="PSUM") as psum,
        ):
            wt_sb = cpool.tile([KAUG, C], bf16)
            nc.sync.dma_start(out=wt_sb[:], in_=wt[:])
            xw_sb = pool.tile([KAUG, DN], bf16)
            # small first piece so the first matmul starts early; the output
            # is written back in pieces to overlap the store with compute
            # (copies stay on DVE/ACT — GPSIMD has no PSUM access path)
            in_offs = [(0, 1024)] + [
                (o, min(1344, DN - o)) for o in range(1024, DN, 1344)
            ]
            for o, w in in_offs:
                nc.sync.dma_start(out=xw_sb[:, o : o + w], in_=xw[:, o : o + w])
            ga_sb = pool.tile([C, DN], bf16)
            nouts, wstep = 0, DN // 4
            for j, off in enumerate(range(0, DN, MM_CH)):
                w = min(MM_CH, DN - off)
                h = psum.tile([C, MM_CH], mybir.dt.float32, tag="h")
                nc.tensor.matmul(
                    out=h[:, 0:w],
                    lhsT=wt_sb[:],
                    rhs=xw_sb[:, off : off + w],
                    start=True,
                    stop=True,
                )
                if j % 2 == 0:
                    nc.vector.tensor_copy(out=ga_sb[:, off : off + w], in_=h[:, 0:w])
                else:
                    nc.scalar.copy(out=ga_sb[:, off : off + w], in_=h[:, 0:w])
                while (nouts + 1) * wstep <= off + w:
                    o = nouts * wstep
                    nc.sync.dma_start(out=ga[:, o : o + wstep], in_=ga_sb[:, o : o + wstep])
                    nouts += 1
    nc.compile()
    return nc


def _halve_max(nc, v, k):
    """In-place log2 max-tree over the slot axis of v [p, t, k, c]; returns
    the [p, t, c] partial-AP at slot 0."""
    while k > 1:
        h = k // 2
        nc.vector.tensor_max(
            out=v[:, :, 0:h, :], in0=v[:, :, 0:h, :], in1=v[:, :, h : 2 * h, :]
        )
        if k % 2:
            nc.vector.tensor_max(
                out=v[:, :, 0:1, :],
                in0=v[:, :, 0:1, :],
                in1=v[:, :, 2 * h : 2 * h + 1, :],
            )
        k = h
    return v[:, :, 0, :]


def _build_gather(chunks, totcols):
    """chunks: list of (t0, T, Ke, Ko, coff_e, coff_o) with uniform slot
    counts per 128-node tile; coff_* are column offsets into the packed idx
    tensor (SBUF layout [32, totcols] int16).  Also recomputes A (grouped
    node order) on the otherwise-idle PE from xwd/wad."""
    import concourse.bacc as bacc
    import concourse.mybir as mybir
    from concourse.tile import TileContext

    nc = bacc.Bacc(
        "TRN2", target_bir_lowering=False, debug=False,
        dynamic_dma_scratch_size=65536,
    )
    bf16 = mybir.dt.bfloat16
    i16 = mybir.dt.int16
    gpair = nc.dram_tensor("gpair", [NPAIR, 2 * C], bf16, kind="ExternalInput")
    idxd = nc.dram_tensor("idxd", [32, totcols], i16, kind="ExternalInput")
    xwd = nc.dram_tensor("xwd", [KAUG, NPAD], bf16, kind="ExternalInput")
    wad = nc.dram_tensor("wad", [KAUG, C], bf16, kind="ExternalInput")
    oshd = nc.dram_tensor("oshd", [P, NT * C], bf16, kind="ExternalOutput")

    # emission order: interleave biggest and smallest chunks so the Pool
    # desc-gen of small (gen-bound) chunks hides behind big transfers
    nsl = lambda ch: ch[1] * (ch[2] + ch[3])
    by_size = sorted(range(len(chunks)), key=lambda i: -nsl(chunks[i]))
    seq = []
    lo_i, hi_i = 0, len(by_size) - 1
    while lo_i <= hi_i:
        seq.append(by_size[lo_i])
        lo_i += 1
        if lo_i <= hi_i:
            seq.append(by_size[hi_i])
            hi_i -= 1
    seq = [chunks[i] for i in seq]

    with TileContext(nc) as tc:
        with (
            tc.tile_pool(name="const", bufs=1) as cpool,
            tc.tile_pool(name="gat", bufs=4) as gpool,
            tc.tile_pool(name="psum", bufs=4, space="PSUM") as psum,
        ):
            idx_sb = cpool.tile([32, totcols], i16)
            # first idx piece covers the first emitted chunk's columns so its
            # desc-gen can start while the bulk loads behind it
            t0, T, ke, ko, ce, co = seq[0]
            lo = min(ce, co)
            hi = max(ce + T * P * ke // 16, co + T * P * ko // 16)
            nc.sync.dma_start(out=idx_sb[:, lo:hi], in_=idxd[:, lo:hi])
            if lo > 0:
                nc.sync.dma_start(out=idx_sb[:, 0:lo], in_=idxd[:, 0:lo])
            if hi < totcols:
                nc.sync.dma_start(out=idx_sb[:, hi:totcols], in_=idxd[:, hi:totcols])
            wa_sb = cpool.tile([KAUG, C], bf16)
            nc.sync.dma_start(out=wa_sb[:], in_=wad[:])
            xw_sb = cpool.tile([KAUG, NPAD], bf16)
            nc.sync.dma_start(out=xw_sb[:], in_=xwd[:])
            ash_sb = cpool.tile([P, NT, C], bf16)
            m_sb = cpool.tile([P, NT, C], bf16)
            o_sb = cpool.tile([P, NT, C], bf16)
            done = [False] * NT
            written = 0
            for ci, (t0, T, ke, ko, coff_e, coff_o) in enumerate(seq):
                # A tiles for this chunk: matmul(lhsT=x_tile_aug, rhs=V1_aug)
                # lands [128 nodes, 64] node-major; copy on the idle ACT engine
                for t in range(t0, t0 + T):
                    ha = psum.tile([P, C], mybir.dt.float32, tag="ha")
                    nc.tensor.matmul(
                        out=ha[:],
                        lhsT=xw_sb[:, t * P : (t + 1) * P],
                        rhs=wa_sb[:],
                        start=True,
                        stop=True,
                    )
                    nc.scalar.copy(out=ash_sb[:, t, :], in_=ha[:])
                parts = []
                for k, coff, tag, base in (
                    (ke, coff_e, "ge", gpair[:, 0:C]),
                    (ko, coff_o, "go", gpair[:, C : 2 * C]),
                ):
                    if k == 0:
                        continue
                    gt = gpool.tile([P, MAXSL, C], bf16, tag=tag)
                    n = T * P * k
                    _raw_gather(
                        nc, gt[:, 0 : T * k, :], base,
                        idx_sb[:, coff : coff + n // 16], n,
                    )
                    v = gt[:, 0 : T * k, :].rearrange("p (t k) c -> p t k c", k=k)
                    parts.append(_halve_max(nc, v, k))
                tgt = m_sb[:, t0 : t0 + T, :]
                if len(parts) == 2:
                    nc.vector.tensor_max(out=tgt, in0=parts[0], in1=parts[1])
                    nc.vector.tensor_add(
                        out=tgt, in0=tgt, in1=ash_sb[:, t0 : t0 + T, :]
                    )
                else:
                    nc.vector.tensor_add(
                        out=tgt, in0=parts[0], in1=ash_sb[:, t0 : t0 + T, :]
                    )
                nc.scalar.activation(
                    out=o_sb[:, t0 : t0 + T, :],
                    in_=tgt,
                    func=mybir.ActivationFunctionType.Relu,
                )
                for t in range(t0, t0 + T):
                    done[t] = True
                pref = NT if all(done) else done.index(False)
                if pref - written >= 12 and ci < len(seq) - 1:
                    # ship the completed prefix while later chunks gather
                    nc.sync.dma_start(
                        out=oshd[:, written * C : pref * C].rearrange(
                            "p (t c) -> p t c", c=C
                        ),
                        in_=o_sb[:, written:pref, :],
                    )
                    written = pref
            nc.sync.dma_start(
                out=oshd[:, written * C : NT * C].rearrange("p (t c) -> p t c", c=C),
                in_=o_sb[:, written:NT, :],
            )
    nc.compile()
    return nc


def _make_layout(src):
    """Host-side grouping: sort each core's nodes by even-source count,
    tile the sorted order into 128-node tiles, assign uniform (Ke, Ko)
    slot counts per run of tiles (shared by all 8 cores), and emit the
    chunked instruction table plus per-core packed idx arrays."""
    orders, es = [], []
    for c in range(N_CORES):
        s = src[c * NSH * DEG : (c + 1) * NSH * DEG].reshape(NSH, DEG)
        e = ((s & 1) == 0).sum(1)
        order = np.argsort(-e, kind="stable")
        orders.append(order)
        es.append(e[order])
    es = np.stack(es)  # [8, NSH] descending per row

    ke_t = [int(es[:, t * P].max()) for t in range(NT)]
    ko_t = [int(DEG - es[:, min(t * P + P, NSH) - 1].min()) for t in range(NT)]

    buckets = []  # [t0, t1, Ke, Ko]
    for t in range(NT):
        if buckets and buckets[-1][2] == ke_t[t] and buckets[-1][3] == ko_t[t]:
            buckets[-1][1] = t + 1
        else:
            buckets.append([t, t + 1, ke_t[t], ko_t[t]])

    def cost(b):
        return (b[1] - b[0]) * (b[2] + b[3])

    while len(buckets) > MAXB:
        best, bi = None, None
        for i in range(len(buckets) - 1):
            a, b = buckets[i], buckets[i + 1]
            add = cost([a[0], b[1], max(a[2], b[2]), max(a[3], b[3])]) - cost(a) - cost(b)
            if best is None or add < best:
                best, bi = add, i
        a, b = buckets[bi], buckets[bi + 1]
        buckets[bi : bi + 2] = [[a[0], b[1], max(a[2], b[2]), max(a[3], b[3])]]

    # split buckets so no gather instruction exceeds MAXSL per-partition slots
    chunks = []  # (t0, T, Ke, Ko, coff_e, coff_o)
    coff = 0
    for t0, t1, ke, ko in buckets:
        step = max(1, MAXSL // max(ke, ko, 1))
        t = t0
        while t < t1:
            T = min(step, t1 - t)
            ce = coff
            coff += T * P * ke // 16
            co = coff
            coff += T * P * ko // 16
            chunks.append((t, T, ke, ko, ce, co))
            t += T
    totcols = coff

    def core_idx(c):
        s = src[c * NSH * DEG : (c + 1) * NSH * DEG].reshape(NSH, DEG)
        ss = s[orders[c]]
        par = ss & 1
        e = (par == 0).sum(1)
        key = np.argsort(par, axis=1, kind="stable")  # evens first
        pr = np.take_along_axis(ss >> 1, key, axis=1).astype(np.int16)
        cols = np.arange(DEG)[None, :]
        ev = np.where(cols < e[:, None], pr, np.int16(SENT_ROW))
        oc = e[:, None] + cols
        od = np.where(
            oc < DEG,
            np.take_along_axis(pr, np.minimum(oc, DEG - 1), axis=1),
            np.int16(SENT_ROW),
        )
        ev_pad = np.full((NPAD, DEG), SENT_ROW, np.int16)
        od_pad = np.full((NPAD, DEG), SENT_ROW, np.int16)
        ev_pad[:NSH] = ev
        od_pad[:NSH] = od
        pieces = []
        for t0, T, ke, ko, _, _ in chunks:
            for arr, k in ((ev_pad, ke), (od_pad, ko)):
                if k == 0:
                    continue
                blk = (
                    arr[t0 * P : (t0 + T) * P, 0:k]
                    .reshape(T, P, k)
                    .transpose(0, 2, 1)
                    .reshape(-1)
                )
                w = blk.reshape(-1, 16).T  # [16, n/16]
                pieces.append(np.tile(w, (2, 1)))  # [32, n/16]
        return np.ascontiguousarray(np.concatenate(pieces, axis=1))

    idx_all = [core_idx(c) for c in range(N_CORES)]
    assert idx_all[0].shape == (32, totcols), (idx_all[0].shape, totcols)
    return orders, chunks, totcols, idx_all


def _numpy_fallback(x, edge_index, W, b):
    src, dst = edge_index[0], edge_index[1]
    V1 = W[:, :C] - W[:, C:]
    V2 = W[:, C:]
    A = x @ V1.T + b
    g = x @ V2.T
    out = np.full((x.shape[0], C), -np.inf, dtype=np.float32)
    msg = np.maximum(A[dst] + g[src], 0.0)
    np.maximum.at(out, dst, msg)
    return np.where(np.isneginf(out), 0.0, out).astype(np.float32)


def _run_spmd(nc, in_maps):
    # the shared axon device occasionally reports a transient
    # NRT_EXEC_UNIT_UNRECOVERABLE on a cold first launch; retry once
    import time
    from concourse.bass_utils import run_bass_kernel_spmd

    try:
        return run_bass_kernel_spmd(nc, in_maps, core_ids=list(range(N_CORES)))
    except Exception:
        time.sleep(10.0)
        return run_bass_kernel_spmd(nc, in_maps, core_ids=list(range(N_CORES)))


def kernel(x, edge_index, edge_attr, W, b):
    bf16 = _bf16()
    x = np.ascontiguousarray(x, dtype=np.float32)
    edge_index = np.ascontiguousarray(edge_index, dtype=np.int32)
    W = np.ascontiguousarray(W, dtype=np.float32)
    b = np.ascontiguousarray(b, dtype=np.float32)

    expected_dst = np.repeat(np.arange(N_NODES, dtype=np.int32), DEG)
    if (
        x.shape != (N_NODES, C)
        or edge_index.shape != (2, N_NODES * DEG)
        or not np.array_equal(edge_index[1], expected_dst)
    ):
        return _numpy_fallback(x, edge_index, W, b)

    src = edge_index[0]
    sig = hash(src.tobytes())
    if _cache.get("layout_sig") != sig:
        _cache["layout"] = _make_layout(src)
        _cache["layout_sig"] = sig
        _cache.pop("gather", None)
    orders, chunks, totcols, idx_all = _cache["layout"]

    if "dense" not in _cache:
        _cache["dense"] = _build_dense()
    if "gather" not in _cache:
        _cache["gather"] = _build_gather(chunks, totcols)

    # ---- Launch 1: dense phase, g = x @ W2.T (transposed layout) ----
    W1, W2 = W[:, :C], W[:, C:]
    wt = np.zeros((KAUG, C), dtype=bf16)
    wt[:C, :] = W2.T.astype(bf16)
    in1 = []
    xts = []
    for c in range(N_CORES):
        xw = np.zeros((KAUG, DN), dtype=bf16)
        xw[:C, :NSH] = x[c * NSH : (c + 1) * NSH].T.astype(bf16)
        xw[C, :] = 1.0
        xts.append(xw)
        in1.append({"xw": xw, "wt": wt})
    r1 = _run_spmd(_cache["dense"], in1)

    g_rows = [
        np.ascontiguousarray(np.asarray(r1.results[c]["ga"])[:, :NSH].T)
        for c in range(N_CORES)
    ]
    g_full = np.concatenate(g_rows, axis=0)  # [N, C] bf16
    gpair = np.concatenate(
        [
            g_full.reshape(NPAIR - 1, 2 * C),
            np.full((1, 2 * C), SENT, dtype=g_full.dtype),
        ],
        axis=0,
    )
    gpair = np.ascontiguousarray(gpair)

    # ---- Launch 2: parity-split gathers + grouped segment max + A ----
    wa = np.zeros((KAUG, C), dtype=bf16)
    wa[:C, :] = (W1 - W2).astype(bf16).T
    wa[C, :] = b.astype(bf16)
    in2 = []
    for c in range(N_CORES):
        xw2 = np.zeros((KAUG, NPAD), dtype=bf16)
        xw2[:, :NSH] = xts[c][:, :NSH][:, orders[c]]
        xw2[C, :] = 1.0
        in2.append({"gpair": gpair, "idxd": idx_all[c], "xwd": xw2, "wad": wa})
    r2 = _run_spmd(_cache["gather"], in2)

    out = np.empty((N_NODES, C), dtype=np.float32)
    for c in range(N_CORES):
        osh = np.asarray(r2.results[c]["oshd"])
        res = osh.reshape(P, NT, C).transpose(1, 0, 2).reshape(NPAD, C)[:NSH]
        out[c * NSH + orders[c]] = res.astype(np.float32)
    _cache["last_results"] = (r1, r2)
    return out


# revision 17
# speedup vs baseline: 1.0129x; 1.0129x over previous
"""EdgeConv (PyG, aggr='max') Trainium2 kernel, 8-core SPMD.

Math: out_i = max_{e: dst(e)=i} relu(x_i @ W1.T + (x_src(e) - x_i) @ W2.T + b)
with W = [W1 | W2].  Rewriting:
    msg_e = relu(A_i + g_src(e)),  A = x @ (W1-W2).T + b,  g = x @ W2.T
Since A_i is constant within segment i and relu is monotone:
    out_i = relu(A_i + max_e g_src(e))
The reference's dst is repeat(arange(N), DEG), so segments are 16 consecutive
edges and the segment-max is a fixed-group reduce after routing each edge's
g row to its slot.

Two SPMD launches on 8 cores:

L1 (dense, node-parallel): host supplies the 6250-node shard pre-transposed,
   xT_aug [65, 6400] bf16, and wt [65, 64] = W2.T (+ zero ones-row).  One
   matmul per 512 nodes emits gT [64, 512] -> bulk store of gT [64, 6400].
   No PE transposes, all DMA transfers are bulk.

L2 (gather + segment max, edge-parallel): the g table is kept as 256B-stride
   pair rows gpair[r] = [g_{2r} | g_{2r+1}] (row 25000 = -3e38 sentinel), but
   each edge's descriptor transfers only the 128B half it needs: dma_gather's
   HBM address math is idx*stride_bytes_256*256 with the transfer size set
   independently by elem_size, so gathers with elem_size=64 bf16 from base
   +0B / +128B views fetch exactly g[src] for even/odd src with idx = src>>1
   (fits int16).  elem_size_bytes%256==0 is only a transpose-mode ucode
   restriction, so the instruction is emitted directly (the bass helper
   over-asserts).  Each node's 16 edges split unevenly between the two
   parity gathers, so the host sorts nodes by even-source count, tiles the
   sorted order into 128-node tiles, and gives each run of tiles uniform
   slot counts (K_even, K_odd) padded with sentinel indices; the segment max
   is then an in-place log2-halving elementwise max over the slot axis.
   A (+ bias, in grouped node order) is recomputed inside this launch on the
   otherwise-idle PE — per 128-node tile, matmul(lhsT=x_tile_aug, rhs=V1_aug)
   lands A directly node-major in PSUM, copied to SBUF on the idle ACT
   engine — then combine = relu(max(even,odd) + A).  All index/permutation
   prep is host-side; the host un-permutes the output rows at the end.
"""

import numpy as np

N_NODES = 50000
DEG = 16
C = 64
N_CORES = 8
NSH = N_NODES // N_CORES  # 6250 nodes per core
P = 128
NT = -(-NSH // P)  # 49 tiles of 128 sorted nodes
NPAD = NT * P  # 6272
NPAIR = N_NODES // 2 + 1  # pair rows + sentinel row
SENT_ROW = N_NODES // 2  # 25000
SENT = -3.0e38
KAUG = C + 1  # x channels + ones row
DN = 6400  # dense-phase padded node count (50 tiles)
MM_CH = 512  # matmul chunk (one PSUM bank)
MAXSL = 64  # max per-partition slots per gather instruction
MAXB = 16  # bucket budget after merging

_cache = {}


def _bf16():
    import ml_dtypes

    return ml_dtypes.bfloat16


def _raw_gather(nc, out_ap, in_ap, idxs_ap, num_idxs):
    """dma_gather with elem_size=64 bf16 (128B payload) on a 256B-stride
    table: bass.dma_gather asserts elem_size_bytes%256==0, but the ucode
    only needs that for transpose mode; emit the instruction directly."""
    import concourse.mybir as mybir

    g = nc.gpsimd
    elem_step = in_ap.ap[0][0]
    stride_bytes = elem_step * mybir.dt.size(in_ap.dtype)
    assert stride_bytes % 256 == 0 and stride_bytes // 256 < 256
    elem_size = in_ap.ap[-1][1]
    assert out_ap.ap[-1][1] == elem_size
    assert out_ap.ap[0][1] * out_ap.ap[1][1] == num_idxs and num_idxs % 128 == 0
    return g.add_instruction(
        mybir.InstDMAGatherAnt(
            name=g.bass.get_next_instruction_name(),
            ins=[
                *g.lower_ap_dma(in_ap, for_custom_bir_dma=True),
                g.lower_ap(idxs_ap),
                g.lower_val_access(g.to_reg(num_idxs)),
            ],
            outs=[g.lower_ap(out_ap)],
            transpose=False,
            num_idxs=num_idxs,
            elem_size=elem_size,
            stride_bytes_256=stride_bytes // 256,
            gen_mode=0,
            single_packet=False,
            queue_num=0,
            sbuf_tokens_per_rank=0,
            sbuf_free_dim_per_rank=0,
            sbuf_free_dim_pad_per_rank=0,
            sbuf_byte_offset=0,
        )
    )


def _build_dense():
    import concourse.bacc as bacc
    import concourse.mybir as mybir
    from concourse.tile import TileContext

    nc = bacc.Bacc("TRN2", target_bir_lowering=False, debug=False)
    bf16 = mybir.dt.bfloat16
    xw = nc.dram_tensor("xw", [KAUG, DN], bf16, kind="ExternalInput")
    wt = nc.dram_tensor("wt", [KAUG, C], bf16, kind="ExternalInput")
    ga = nc.dram_tensor("ga", [C, DN], bf16, kind="ExternalOutput")

    with TileContext(nc) as tc:
        with (
            tc.tile_pool(name="const", bufs=1) as cpool,
            tc.tile_pool(name="sbuf", bufs=1) as pool,
            tc.tile_pool(name="psum", bufs=4, space="PSUM") as psum,
        ):
            wt_sb = cpool.tile([KAUG, C], bf16)
            nc.sync.dma_start(out=wt_sb[:], in_=wt[:])
            xw_sb = pool.tile([KAUG, DN], bf16)
            # small first piece so the first matmul starts early; the output
            # is written back in pieces to overlap the store with compute
            # (copies stay on DVE/ACT — GPSIMD has no PSUM access path)
            in_offs = [(0, 1024)] + [
                (o, min(1344, DN - o)) for o in range(1024, DN, 1344)
            ]
            for o, w in in_offs:
                nc.sync.dma_start(out=xw_sb[:, o : o + w], in_=xw[:, o : o + w])
            ga_sb = pool.tile([C, DN], bf16)
            nouts, wstep = 0, DN // 4
            for j, off in enumerate(range(0, DN, MM_CH)):
                w = min(MM_CH, DN - off)
                h = psum.tile([C, MM_CH], mybir.dt.float32, tag="h")
                nc.tensor.matmul(
                    out=h[:, 0:w],
                    lhsT=wt_sb[:],
                    rhs=xw_sb[:, off : off + w],
                    start=True,
                    stop=True,
                )
                if j % 2 == 0:
                    nc.vector.tensor_copy(out=ga_sb[:, off : off + w], in_=h[:, 0:w])
                else:
                    nc.scalar.copy(out=ga_sb[:, off : off + w], in_=h[:, 0:w])
                while (nouts + 1) * wstep <= off + w:
                    o = nouts * wstep
                    nc.sync.dma_start(out=ga[:, o : o + wstep], in_=ga_sb[:, o : o + wstep])
                    nouts += 1
    nc.compile()
    return nc


def _halve_max(nc, v, k):
    """In-place log2 max-tree over the slot axis of v [p, t, k, c]; returns
    the [p, t, c] partial-AP at slot 0."""
    while k > 1:
        h = k // 2
        nc.vector.tensor_max(
            out=v[:, :, 0:h, :], in0=v[:, :, 0:h, :], in1=v[:, :, h : 2 * h, :]
        )
        if k % 2:
            nc.vector.tensor_max(
                out=v[:, :, 0:1, :],
                in0=v[:, :, 0:1, :],
                in1=v[:, :, 2 * h : 2 * h + 1, :],
            )
        k = h
    return v[:, :, 0, :]


def _build_gather(chunks, totcols):
    """chunks: list of (t0, T, Ke, Ko, coff_e, coff_o) with uniform slot
    counts per 128-node tile; coff_* are column offsets into the packed idx
    tensor (SBUF layout [32, totcols] int16).  Also recomputes A (grouped
    node order) on the otherwise-idle PE from xwd/wad."""
    import concourse.bacc as bacc
    import concourse.mybir as mybir
    from concourse.tile import TileContext

    nc = bacc.Bacc(
        "TRN2", target_bir_lowering=False, debug=False,
        dynamic_dma_scratch_size=65536,
    )
    bf16 = mybir.dt.bfloat16
    i16 = mybir.dt.int16
    gpair = nc.dram_tensor("gpair", [NPAIR, 2 * C], bf16, kind="ExternalInput")
    idxd = nc.dram_tensor("idxd", [32, totcols], i16, kind="ExternalInput")
    xwd = nc.dram_tensor("xwd", [KAUG, NPAD], bf16, kind="ExternalInput")
    wad = nc.dram_tensor("wad", [KAUG, C], bf16, kind="ExternalInput")
    oshd = nc.dram_tensor("oshd", [P, NT * C], bf16, kind="ExternalOutput")

    # emission order: interleave biggest and smallest chunks so the Pool
    # desc-gen of small (gen-bound) chunks hides behind big transfers
    nsl = lambda ch: ch[1] * (ch[2] + ch[3])
    by_size = sorted(range(len(chunks)), key=lambda i: -nsl(chunks[i]))
    seq = []
    lo_i, hi_i = 0, len(by_size) - 1
    while lo_i <= hi_i:
        seq.append(by_size[lo_i])
        lo_i += 1
        if lo_i <= hi_i:
            seq.append(by_size[hi_i])
            hi_i -= 1
    seq = [chunks[i] for i in seq]

    with TileContext(nc) as tc:
        with (
            tc.tile_pool(name="const", bufs=1) as cpool,
            tc.tile_pool(name="gat", bufs=4) as gpool,
            tc.tile_pool(name="psum", bufs=4, space="PSUM") as psum,
        ):
            idx_sb = cpool.tile([32, totcols], i16)
            # first idx piece covers the first emitted chunk's columns so its
            # desc-gen can start while the bulk loads behind it
            t0, T, ke, ko, ce, co = seq[0]
            lo = min(ce, co)
            hi = max(ce + T * P * ke // 16, co + T * P * ko // 16)
            nc.sync.dma_start(out=idx_sb[:, lo:hi], in_=idxd[:, lo:hi])
            if lo > 0:
                nc.sync.dma_start(out=idx_sb[:, 0:lo], in_=idxd[:, 0:lo])
            if hi < totcols:
                nc.sync.dma_start(out=idx_sb[:, hi:totcols], in_=idxd[:, hi:totcols])
            wa_sb = cpool.tile([KAUG, C], bf16)
            nc.sync.dma_start(out=wa_sb[:], in_=wad[:])
            xw_sb = cpool.tile([KAUG, NPAD], bf16)
            nc.sync.dma_start(out=xw_sb[:], in_=xwd[:])
            ash_sb = cpool.tile([P, NT, C], bf16)
            m_sb = cpool.tile([P, NT, C], bf16)
            o_sb = cpool.tile([P, NT, C], bf16)
            done = [False] * NT
            written = [False] * NT

            def flush(minrun):
                # ship every completed-but-unwritten run of >= minrun tiles
                t = 0
                while t < NT:
                    if done[t] and not written[t]:
                        u = t
                        while u < NT and done[u] and not written[u]:
                            u += 1
                        if u - t >= minrun:
                            nc.sync.dma_start(
                                out=oshd[:, t * C : u * C].rearrange(
                                    "p (t c) -> p t c", c=C
                                ),
                                in_=o_sb[:, t:u, :],
                            )
                            for q in range(t, u):
                                written[q] = True
                        t = u
                    else:
                        t += 1

            for ci, (t0, T, ke, ko, coff_e, coff_o) in enumerate(seq):
                # A tiles for this chunk: matmul(lhsT=x_tile_aug, rhs=V1_aug)
                # lands [128 nodes, 64] node-major; copy on the idle ACT engine
                for t in range(t0, t0 + T):
                    ha = psum.tile([P, C], mybir.dt.float32, tag="ha")
                    nc.tensor.matmul(
                        out=ha[:],
                        lhsT=xw_sb[:, t * P : (t + 1) * P],
                        rhs=wa_sb[:],
                        start=True,
                        stop=True,
                    )
                    nc.scalar.copy(out=ash_sb[:, t, :], in_=ha[:])
                parts = []
                for k, coff, tag, base in (
                    (ke, coff_e, "ge", gpair[:, 0:C]),
                    (ko, coff_o, "go", gpair[:, C : 2 * C]),
                ):
                    if k == 0:
                        continue
                    gt = gpool.tile([P, MAXSL, C], bf16, tag=tag)
                    n = T * P * k
                    _raw_gather(
                        nc, gt[:, 0 : T * k, :], base,
                        idx_sb[:, coff : coff + n // 16], n,
                    )
                    v = gt[:, 0 : T * k, :].rearrange("p (t k) c -> p t k c", k=k)
                    parts.append(_halve_max(nc, v, k))
                tgt = m_sb[:, t0 : t0 + T, :]
                if len(parts) == 2:
                    nc.vector.tensor_max(out=tgt, in0=parts[0], in1=parts[1])
                    nc.vector.tensor_add(
                        out=tgt, in0=tgt, in1=ash_sb[:, t0 : t0 + T, :]
                    )
                else:
                    nc.vector.tensor_add(
                        out=tgt, in0=parts[0], in1=ash_sb[:, t0 : t0 + T, :]
                    )
                if ci == len(seq) - 1:
                    # keep the final chunk's chain on DVE — skips the
                    # cross-engine hop on the tail critical path
                    nc.vector.tensor_scalar_max(
                        out=o_sb[:, t0 : t0 + T, :], in0=tgt, scalar1=0.0
                    )
                else:
                    nc.scalar.activation(
                        out=o_sb[:, t0 : t0 + T, :],
                        in_=tgt,
                        func=mybir.ActivationFunctionType.Relu,
                    )
                for t in range(t0, t0 + T):
                    done[t] = True
                pref = NT if all(done) else done.index(False)
                if (
                    sum(done) - sum(written) >= 14
                    and ci < len(seq) - 1
                    and pref - sum(written) >= 12
                ):
                    flush(12)
            flush(1)
    nc.compile()
    return nc


def _make_layout(src):
    """Host-side grouping: sort each core's nodes by even-source count,
    tile the sorted order into 128-node tiles, assign uniform (Ke, Ko)
    slot counts per run of tiles (shared by all 8 cores), and emit the
    chunked instruction table plus per-core packed idx arrays."""
    orders, es = [], []
    for c in range(N_CORES):
        s = src[c * NSH * DEG : (c + 1) * NSH * DEG].reshape(NSH, DEG)
        e = ((s & 1) == 0).sum(1)
        order = np.argsort(-e, kind="stable")
        orders.append(order)
        es.append(e[order])
    es = np.stack(es)  # [8, NSH] descending per row

    ke_t = [int(es[:, t * P].max()) for t in range(NT)]
    ko_t = [int(DEG - es[:, min(t * P + P, NSH) - 1].min()) for t in range(NT)]

    buckets = []  # [t0, t1, Ke, Ko]
    for t in range(NT):
        if buckets and buckets[-1][2] == ke_t[t] and buckets[-1][3] == ko_t[t]:
            buckets[-1][1] = t + 1
        else:
            buckets.append([t, t + 1, ke_t[t], ko_t[t]])

    def cost(b):
        return (b[1] - b[0]) * (b[2] + b[3])

    while len(buckets) > MAXB:
        best, bi = None, None
        for i in range(len(buckets) - 1):
            a, b = buckets[i], buckets[i + 1]
            add = cost([a[0], b[1], max(a[2], b[2]), max(a[3], b[3])]) - cost(a) - cost(b)
            if best is None or add < best:
                best, bi = add, i
        a, b = buckets[bi], buckets[bi + 1]
        buckets[bi : bi + 2] = [[a[0], b[1], max(a[2], b[2]), max(a[3], b[3])]]

    # split buckets so no gather instruction exceeds MAXSL per-partition slots
    chunks = []  # (t0, T, Ke, Ko, coff_e, coff_o)
    coff = 0
    for t0, t1, ke, ko in buckets:
        step = max(1, MAXSL // max(ke, ko, 1))
        t = t0
        while t < t1:
            T = min(step, t1 - t)
            ce = coff
            coff += T * P * ke // 16
            co = coff
            coff += T * P * ko // 16
            chunks.append((t, T, ke, ko, ce, co))
            t += T
    totcols = coff

    def core_idx(c):
        s = src[c * NSH * DEG : (c + 1) * NSH * DEG].reshape(NSH, DEG)
        ss = s[orders[c]]
        par = ss & 1
        e = (par == 0).sum(1)
        key = np.argsort(par, axis=1, kind="stable")  # evens first
        pr = np.take_along_axis(ss >> 1, key, axis=1).astype(np.int16)
        cols = np.arange(DEG)[None, :]
        ev = np.where(cols < e[:, None], pr, np.int16(SENT_ROW))
        oc = e[:, None] + cols
        od = np.where(
            oc < DEG,
            np.take_along_axis(pr, np.minimum(oc, DEG - 1), axis=1),
            np.int16(SENT_ROW),
        )
        ev_pad = np.full((NPAD, DEG), SENT_ROW, np.int16)
        od_pad = np.full((NPAD, DEG), SENT_ROW, np.int16)
        ev_pad[:NSH] = ev
        od_pad[:NSH] = od
        pieces = []
        for t0, T, ke, ko, _, _ in chunks:
            for arr, k in ((ev_pad, ke), (od_pad, ko)):
                if k == 0:
                    continue
                blk = (
                    arr[t0 * P : (t0 + T) * P, 0:k]
                    .reshape(T, P, k)
                    .transpose(0, 2, 1)
                    .reshape(-1)
                )
                w = blk.reshape(-1, 16).T  # [16, n/16]
                pieces.append(np.tile(w, (2, 1)))  # [32, n/16]
        return np.ascontiguousarray(np.concatenate(pieces, axis=1))

    idx_all = [core_idx(c) for c in range(N_CORES)]
    assert idx_all[0].shape == (32, totcols), (idx_all[0].shape, totcols)
    return orders, chunks, totcols, idx_all


def _numpy_fallback(x, edge_index, W, b):
    src, dst = edge_index[0], edge_index[1]
    V1 = W[:, :C] - W[:, C:]
    V2 = W[:, C:]
    A = x @ V1.T + b
    g = x @ V2.T
    out = np.full((x.shape[0], C), -np.inf, dtype=np.float32)
    msg = np.maximum(A[dst] + g[src], 0.0)
    np.maximum.at(out, dst, msg)
    return np.where(np.isneginf(out), 0.0, out).astype(np.float32)


def _run_spmd(nc, in_maps):
    # the shared axon device occasionally reports a transient
    # NRT_EXEC_UNIT_UNRECOVERABLE on a cold first launch; retry once
    import time
    from concourse.bass_utils import run_bass_kernel_spmd

    try:
        return run_bass_kernel_spmd(nc, in_maps, core_ids=list(range(N_CORES)))
    except Exception:
        time.sleep(10.0)
        return run_bass_kernel_spmd(nc, in_maps, core_ids=list(range(N_CORES)))


def kernel(x, edge_index, edge_attr, W, b):
    bf16 = _bf16()
    x = np.ascontiguousarray(x, dtype=np.float32)
    edge_index = np.ascontiguousarray(edge_index, dtype=np.int32)
    W = np.ascontiguousarray(W, dtype=np.float32)
    b = np.ascontiguousarray(b, dtype=np.float32)

    expected_dst = np.repeat(np.arange(N_NODES, dtype=np.int32), DEG)
    if (
        x.shape != (N_NODES, C)
        or edge_index.shape != (2, N_NODES * DEG)
        or not np.array_equal(edge_index[1], expected_dst)
    ):
        return _numpy_fallback(x, edge_index, W, b)

    src = edge_index[0]
    sig = hash(src.tobytes())
    if _cache.get("layout_sig") != sig:
        _cache["layout"] = _make_layout(src)
        _cache["layout_sig"] = sig
        _cache.pop("gather", None)
    orders, chunks, totcols, idx_all = _cache["layout"]

    if "dense" not in _cache:
        _cache["dense"] = _build_dense()
    if "gather" not in _cache:
        _cache["gather"] = _build_gather(chunks, totcols)

    # ---- Launch 1: dense phase, g = x @ W2.T (transposed layout) ----
    W1, W2 = W[:, :C], W[:, C:]
    wt = np.zeros((KAUG, C), dtype=bf16)
    wt[:C, :] = W2.T.astype(bf16)
    in1 = []
    xts = []
    for c in range(N_CORES):
        xw = np.zeros((KAUG, DN), dtype=bf16)
        xw[:C, :NSH] = x[c * NSH : (c + 1) * NSH].T.astype(bf16)
        xw[C, :] = 1.0
        xts.append(xw)
        in1.append({"xw": xw, "wt": wt})
    r1 = _run_spmd(_cache["dense"], in1)

    g_rows = [
        np.ascontiguousarray(np.asarray(r1.results[c]["ga"])[:, :NSH].T)
        for c in range(N_CORES)
    ]
    g_full = np.concatenate(g_rows, axis=0)  # [N, C] bf16
    gpair = np.concatenate(
        [
            g_full.reshape(NPAIR - 1, 2 * C),
            np.full((1, 2 * C), SENT, dtype=g_full.dtype),
        ],
        axis=0,
    )
    gpair = np.ascontiguousarray(gpair)

    # ---- Launch 2: parity-split gathers + grouped segment max + A ----
    wa = np.zeros((KAUG, C), dtype=bf16)
    wa[:C, :] = (W1 - W2).astype(bf16).T
    wa[C, :] = b.astype(bf16)
    in2 = []
    for c in range(N_CORES):
        xw2 = np.zeros((KAUG, NPAD), dtype=bf16)
        xw2[:, :NSH] = xts[c][:, :NSH][:, orders[c]]
        xw2[C, :] = 1.0
        in2.append({"gpair": gpair, "idxd": idx_all[c], "xwd": xw2, "wad": wa})
    r2 = _run_spmd(_cache["gather"], in2)

    out = np.empty((N_NODES, C), dtype=np.float32)
    for c in range(N_CORES):
        osh = np.asarray(r2.results[c]["oshd"])
        res = osh.reshape(P, NT, C).transpose(1, 0, 2).reshape(NPAD, C)[:NSH]
        out[c * NSH + orders[c]] = res.astype(np.float32)
    _cache["last_results"] = (r1, r2)
    return out


# revision 22
# speedup vs baseline: 1.0225x; 1.0095x over previous
"""EdgeConv (PyG, aggr='max') Trainium2 kernel, 8-core SPMD.

Math: out_i = max_{e: dst(e)=i} relu(x_i @ W1.T + (x_src(e) - x_i) @ W2.T + b)
with W = [W1 | W2].  Rewriting:
    msg_e = relu(A_i + g_src(e)),  A = x @ (W1-W2).T + b,  g = x @ W2.T
Since A_i is constant within segment i and relu is monotone:
    out_i = relu(A_i + max_e g_src(e))
The reference's dst is repeat(arange(N), DEG), so segments are 16 consecutive
edges and the segment-max is a fixed-group reduce after routing each edge's
g row to its slot.

Two SPMD launches on 8 cores:

L1 (dense, node-parallel): host supplies the 6250-node shard pre-transposed,
   xT_aug [65, 6400] bf16, and wt [65, 64] = W2.T (+ zero ones-row).  One
   matmul per 512 nodes emits gT [64, 512] -> bulk store of gT [64, 6400].
   No PE transposes, all DMA transfers are bulk.

L2 (gather + segment max, edge-parallel): the g table is kept as 256B-stride
   pair rows gpair[r] = [g_{2r} | g_{2r+1}] (row 25000 = -3e38 sentinel), but
   each edge's descriptor transfers only the 128B half it needs: dma_gather's
   HBM address math is idx*stride_bytes_256*256 with the transfer size set
   independently by elem_size, so gathers with elem_size=64 bf16 from base
   +0B / +128B views fetch exactly g[src] for even/odd src with idx = src>>1
   (fits int16).  elem_size_bytes%256==0 is only a transpose-mode ucode
   restriction, so the instruction is emitted directly (the bass helper
   over-asserts).  Each node's 16 edges split unevenly between the two
   parity gathers, so the host sorts nodes by even-source count, tiles the
   sorted order into 128-node tiles, and gives each run of tiles uniform
   slot counts (K_even, K_odd) padded with sentinel indices; the segment max
   is then an in-place log2-halving elementwise max over the slot axis.
   A (+ bias, in grouped node order) is recomputed inside this launch on the
   otherwise-idle PE — per 128-node tile, matmul(lhsT=x_tile_aug, rhs=V1_aug)
   lands A directly node-major in PSUM, copied to SBUF on the idle ACT
   engine — then combine = relu(max(even,odd) + A).  All index/permutation
   prep is host-side; the host un-permutes the output rows at the end.
"""

import numpy as np

N_NODES = 50000
DEG = 16
C = 64
N_CORES = 8
NSH = N_NODES // N_CORES  # 6250 nodes per core
P = 128
NT = -(-NSH // P)  # 49 tiles of 128 sorted nodes
NPAD = NT * P  # 6272
NPAIR = N_NODES // 2 + 1  # pair rows + sentinel row
SENT_ROW = N_NODES // 2  # 25000
SENT = -3.0e38
KAUG = C + 1  # x channels + ones row
DN = 6400  # dense-phase padded node count (50 tiles)
MM_CH = 512  # matmul chunk (one PSUM bank)
MAXSL = 64  # max per-partition slots per gather instruction
MAXB = 16  # bucket budget after merging

_cache = {}


def _bf16():
    import ml_dtypes

    return ml_dtypes.bfloat16


def _raw_gather(nc, out_ap, in_ap, idxs_ap, num_idxs):
    """dma_gather with elem_size=64 bf16 (128B payload) on a 256B-stride
    table: bass.dma_gather asserts elem_size_bytes%256==0, but the ucode
    only needs that for transpose mode; emit the instruction directly."""
    import concourse.mybir as mybir

    g = nc.gpsimd
    elem_step = in_ap.ap[0][0]
    stride_bytes = elem_step * mybir.dt.size(in_ap.dtype)
    assert stride_bytes % 256 == 0 and stride_bytes // 256 < 256
    elem_size = in_ap.ap[-1][1]
    assert out_ap.ap[-1][1] == elem_size
    assert out_ap.ap[0][1] * out_ap.ap[1][1] == num_idxs and num_idxs % 128 == 0
    return g.add_instruction(
        mybir.InstDMAGatherAnt(
            name=g.bass.get_next_instruction_name(),
            ins=[
                *g.lower_ap_dma(in_ap, for_custom_bir_dma=True),
                g.lower_ap(idxs_ap),
                g.lower_val_access(g.to_reg(num_idxs)),
            ],
            outs=[g.lower_ap(out_ap)],
            transpose=False,
            num_idxs=num_idxs,
            elem_size=elem_size,
            stride_bytes_256=stride_bytes // 256,
            gen_mode=0,
            single_packet=False,
            queue_num=0,
            sbuf_tokens_per_rank=0,
            sbuf_free_dim_per_rank=0,
            sbuf_free_dim_pad_per_rank=0,
            sbuf_byte_offset=0,
        )
    )


def _build_dense():
    import concourse.bacc as bacc
    import concourse.mybir as mybir
    from concourse.tile import TileContext

    nc = bacc.Bacc("TRN2", target_bir_lowering=False, debug=False)
    bf16 = mybir.dt.bfloat16
    xw = nc.dram_tensor("xw", [KAUG, DN], bf16, kind="ExternalInput")
    wt = nc.dram_tensor("wt", [KAUG, C], bf16, kind="ExternalInput")
    ga = nc.dram_tensor("ga", [C, DN], bf16, kind="ExternalOutput")

    with TileContext(nc) as tc:
        with (
            tc.tile_pool(name="const", bufs=1) as cpool,
            tc.tile_pool(name="sbuf", bufs=1) as pool,
            tc.tile_pool(name="psum", bufs=4, space="PSUM") as psum,
        ):
            wt_sb = cpool.tile([KAUG, C], bf16)
            # tiny weight load rides the idle Pool DMA path, keeping the
            # exclusive HWDGE queue free for the xw pieces
            nc.gpsimd.dma_start(out=wt_sb[:], in_=wt[:])
            xw_sb = pool.tile([KAUG, DN], bf16)
            # small first piece so the first matmul starts early; the output
            # is written back in pieces to overlap the store with compute
            # (copies stay on DVE/ACT — GPSIMD has no PSUM access path)
            in_offs = [(0, 1024)] + [
                (o, min(1344, DN - o)) for o in range(1024, DN, 1344)
            ]
            for o, w in in_offs:
                nc.sync.dma_start(out=xw_sb[:, o : o + w], in_=xw[:, o : o + w])
            ga_sb = pool.tile([C, DN], bf16)
            nouts, wstep = 0, DN // 4
            for j, off in enumerate(range(0, DN, MM_CH)):
                w = min(MM_CH, DN - off)
                h = psum.tile([C, MM_CH], mybir.dt.float32, tag="h")
                nc.tensor.matmul(
                    out=h[:, 0:w],
                    lhsT=wt_sb[:],
                    rhs=xw_sb[:, off : off + w],
                    start=True,
                    stop=True,
                )
                if j % 2 == 0:
                    nc.vector.tensor_copy(out=ga_sb[:, off : off + w], in_=h[:, 0:w])
                else:
                    nc.scalar.copy(out=ga_sb[:, off : off + w], in_=h[:, 0:w])
                while (nouts + 1) * wstep <= off + w:
                    o = nouts * wstep
                    eng = nc.gpsimd if nouts == 0 else nc.sync
                    eng.dma_start(out=ga[:, o : o + wstep], in_=ga_sb[:, o : o + wstep])
                    nouts += 1
    nc.compile()
    return nc


def _halve_max(nc, v, k):
    """In-place log2 max-tree over the slot axis of v [p, t, k, c]; returns
    the [p, t, c] partial-AP at slot 0."""
    while k > 1:
        h = k // 2
        nc.vector.tensor_max(
            out=v[:, :, 0:h, :], in0=v[:, :, 0:h, :], in1=v[:, :, h : 2 * h, :]
        )
        if k % 2:
            nc.vector.tensor_max(
                out=v[:, :, 0:1, :],
                in0=v[:, :, 0:1, :],
                in1=v[:, :, 2 * h : 2 * h + 1, :],
            )
        k = h
    return v[:, :, 0, :]


def _build_gather(chunks, totcols):
    """chunks: list of (t0, T, Ke, Ko, coff_e, coff_o) with uniform slot
    counts per 128-node tile; coff_* are column offsets into the packed idx
    tensor (SBUF layout [32, totcols] int16).  Also recomputes A (grouped
    node order) on the otherwise-idle PE from xwd/wad."""
    import concourse.bacc as bacc
    import concourse.mybir as mybir
    from concourse.tile import TileContext

    nc = bacc.Bacc(
        "TRN2", target_bir_lowering=False, debug=False,
        dynamic_dma_scratch_size=65536,
    )
    bf16 = mybir.dt.bfloat16
    i16 = mybir.dt.int16
    gpair = nc.dram_tensor("gpair", [NPAIR, 2 * C], bf16, kind="ExternalInput")
    idxd = nc.dram_tensor("idxd", [32, totcols], i16, kind="ExternalInput")
    xwd = nc.dram_tensor("xwd", [KAUG, NPAD], bf16, kind="ExternalInput")
    wad = nc.dram_tensor("wad", [KAUG, C], bf16, kind="ExternalInput")
    oshd = nc.dram_tensor("oshd", [P, NT * C], bf16, kind="ExternalOutput")

    # emission order: interleave biggest and smallest chunks so the Pool
    # desc-gen of small (gen-bound) chunks hides behind big transfers
    nsl = lambda ch: ch[1] * (ch[2] + ch[3])
    by_size = sorted(range(len(chunks)), key=lambda i: -nsl(chunks[i]))
    seq = []
    lo_i, hi_i = 0, len(by_size) - 1
    while lo_i <= hi_i:
        seq.append(by_size[lo_i])
        lo_i += 1
        if lo_i <= hi_i:
            seq.append(by_size[hi_i])
            hi_i -= 1
    seq = [chunks[i] for i in seq]

    with TileContext(nc) as tc:
        with (
            tc.tile_pool(name="const", bufs=1) as cpool,
            tc.tile_pool(name="gat", bufs=4) as gpool,
            tc.tile_pool(name="psum", bufs=4, space="PSUM") as psum,
        ):
            idx_sb = cpool.tile([32, totcols], i16)
            # first idx piece covers the first emitted chunk's columns so its
            # desc-gen can start while the bulk loads behind it
            t0, T, ke, ko, ce, co = seq[0]
            lo = min(ce, co)
            hi = max(ce + T * P * ke // 16, co + T * P * ko // 16)
            nc.sync.dma_start(out=idx_sb[:, lo:hi], in_=idxd[:, lo:hi])
            if lo > 0:
                nc.sync.dma_start(out=idx_sb[:, 0:lo], in_=idxd[:, 0:lo])
            if hi < totcols:
                nc.sync.dma_start(out=idx_sb[:, hi:totcols], in_=idxd[:, hi:totcols])
            wa_sb = cpool.tile([KAUG, C], bf16)
            nc.sync.dma_start(out=wa_sb[:], in_=wad[:])
            xw_sb = cpool.tile([KAUG, NPAD], bf16)
            nc.sync.dma_start(out=xw_sb[:], in_=xwd[:])
            ash_sb = cpool.tile([P, NT, C], bf16)
            m_sb = cpool.tile([P, NT, C], bf16)
            o_sb = cpool.tile([P, NT, C], bf16)
            done = [False] * NT
            written = [False] * NT

            def flush(minrun, eng=None):
                # ship every completed-but-unwritten run of >= minrun tiles
                t = 0
                while t < NT:
                    if done[t] and not written[t]:
                        u = t
                        while u < NT and done[u] and not written[u]:
                            u += 1
                        if u - t >= minrun:
                            (eng or nc.sync).dma_start(
                                out=oshd[:, t * C : u * C].rearrange(
                                    "p (t c) -> p t c", c=C
                                ),
                                in_=o_sb[:, t:u, :],
                            )
                            for q in range(t, u):
                                written[q] = True
                        t = u
                    else:
                        t += 1

            for ci, (t0, T, ke, ko, coff_e, coff_o) in enumerate(seq):
                # A tiles for this chunk: matmul(lhsT=x_tile_aug, rhs=V1_aug)
                # lands [128 nodes, 64] node-major; copy on the idle ACT engine
                for t in range(t0, t0 + T):
                    ha = psum.tile([P, C], mybir.dt.float32, tag="ha")
                    nc.tensor.matmul(
                        out=ha[:],
                        lhsT=xw_sb[:, t * P : (t + 1) * P],
                        rhs=wa_sb[:],
                        start=True,
                        stop=True,
                    )
                    nc.scalar.copy(out=ash_sb[:, t, :], in_=ha[:])
                parts = []
                for k, coff, tag, base in (
                    (ke, coff_e, "ge", gpair[:, 0:C]),
                    (ko, coff_o, "go", gpair[:, C : 2 * C]),
                ):
                    if k == 0:
                        continue
                    gt = gpool.tile([P, MAXSL, C], bf16, tag=tag)
                    n = T * P * k
                    _raw_gather(
                        nc, gt[:, 0 : T * k, :], base,
                        idx_sb[:, coff : coff + n // 16], n,
                    )
                    v = gt[:, 0 : T * k, :].rearrange("p (t k) c -> p t k c", k=k)
                    parts.append(_halve_max(nc, v, k))
                tgt = m_sb[:, t0 : t0 + T, :]
                if len(parts) == 2:
                    nc.vector.tensor_max(out=tgt, in0=parts[0], in1=parts[1])
                    nc.vector.tensor_add(
                        out=tgt, in0=tgt, in1=ash_sb[:, t0 : t0 + T, :]
                    )
                else:
                    nc.vector.tensor_add(
                        out=tgt, in0=parts[0], in1=ash_sb[:, t0 : t0 + T, :]
                    )
                if ci == len(seq) - 1:
                    # keep the final chunk's chain on DVE — skips the
                    # cross-engine hop on the tail critical path
                    nc.vector.tensor_scalar_max(
                        out=o_sb[:, t0 : t0 + T, :], in0=tgt, scalar1=0.0
                    )
                else:
                    nc.scalar.activation(
                        out=o_sb[:, t0 : t0 + T, :],
                        in_=tgt,
                        func=mybir.ActivationFunctionType.Relu,
                    )
                for t in range(t0, t0 + T):
                    done[t] = True
                pref = NT if all(done) else done.index(False)
                if (
                    sum(done) - sum(written) >= 14
                    and ci < len(seq) - 1
                    and pref - sum(written) >= 12
                ):
                    flush(12)
            flush(1)
    nc.compile()
    return nc


def _make_layout(src):
    """Host-side grouping: sort each core's nodes by even-source count,
    tile the sorted order into 128-node tiles, assign uniform (Ke, Ko)
    slot counts per run of tiles (shared by all 8 cores), and emit the
    chunked instruction table plus per-core packed idx arrays."""
    orders, es = [], []
    for c in range(N_CORES):
        s = src[c * NSH * DEG : (c + 1) * NSH * DEG].reshape(NSH, DEG)
        e = ((s & 1) == 0).sum(1)
        order = np.argsort(-e, kind="stable")
        orders.append(order)
        es.append(e[order])
    es = np.stack(es)  # [8, NSH] descending per row

    ke_t = [int(es[:, t * P].max()) for t in range(NT)]
    ko_t = [int(DEG - es[:, min(t * P + P, NSH) - 1].min()) for t in range(NT)]

    buckets = []  # [t0, t1, Ke, Ko]
    for t in range(NT):
        if buckets and buckets[-1][2] == ke_t[t] and buckets[-1][3] == ko_t[t]:
            buckets[-1][1] = t + 1
        else:
            buckets.append([t, t + 1, ke_t[t], ko_t[t]])

    def cost(b):
        return (b[1] - b[0]) * (b[2] + b[3])

    while len(buckets) > MAXB:
        best, bi = None, None
        for i in range(len(buckets) - 1):
            a, b = buckets[i], buckets[i + 1]
            add = cost([a[0], b[1], max(a[2], b[2]), max(a[3], b[3])]) - cost(a) - cost(b)
            if best is None or add < best:
                best, bi = add, i
        a, b = buckets[bi], buckets[bi + 1]
        buckets[bi : bi + 2] = [[a[0], b[1], max(a[2], b[2]), max(a[3], b[3])]]

    # split buckets so no gather instruction exceeds MAXSL per-partition slots
    chunks = []  # (t0, T, Ke, Ko, coff_e, coff_o)
    coff = 0
    for t0, t1, ke, ko in buckets:
        step = max(1, MAXSL // max(ke, ko, 1))
        t = t0
        while t < t1:
            T = min(step, t1 - t)
            ce = coff
            coff += T * P * ke // 16
            co = coff
            coff += T * P * ko // 16
            chunks.append((t, T, ke, ko, ce, co))
            t += T
    totcols = coff

    def core_idx(c):
        s = src[c * NSH * DEG : (c + 1) * NSH * DEG].reshape(NSH, DEG)
        ss = s[orders[c]]
        par = ss & 1
        e = (par == 0).sum(1)
        key = np.argsort(par, axis=1, kind="stable")  # evens first
        pr = np.take_along_axis(ss >> 1, key, axis=1).astype(np.int16)
        cols = np.arange(DEG)[None, :]
        ev = np.where(cols < e[:, None], pr, np.int16(SENT_ROW))
        oc = e[:, None] + cols
        od = np.where(
            oc < DEG,
            np.take_along_axis(pr, np.minimum(oc, DEG - 1), axis=1),
            np.int16(SENT_ROW),
        )
        ev_pad = np.full((NPAD, DEG), SENT_ROW, np.int16)
        od_pad = np.full((NPAD, DEG), SENT_ROW, np.int16)
        ev_pad[:NSH] = ev
        od_pad[:NSH] = od
        pieces = []
        for t0, T, ke, ko, _, _ in chunks:
            for arr, k in ((ev_pad, ke), (od_pad, ko)):
                if k == 0:
                    continue
                blk = (
                    arr[t0 * P : (t0 + T) * P, 0:k]
                    .reshape(T, P, k)
                    .transpose(0, 2, 1)
                    .reshape(-1)
                )
                w = blk.reshape(-1, 16).T  # [16, n/16]
                pieces.append(np.tile(w, (2, 1)))  # [32, n/16]
        return np.ascontiguousarray(np.concatenate(pieces, axis=1))

    idx_all = [core_idx(c) for c in range(N_CORES)]
    assert idx_all[0].shape == (32, totcols), (idx_all[0].shape, totcols)
    return orders, chunks, totcols, idx_all


def _numpy_fallback(x, edge_index, W, b):
    src, dst = edge_index[0], edge_index[1]
    V1 = W[:, :C] - W[:, C:]
    V2 = W[:, C:]
    A = x @ V1.T + b
    g = x @ V2.T
    out = np.full((x.shape[0], C), -np.inf, dtype=np.float32)
    msg = np.maximum(A[dst] + g[src], 0.0)
    np.maximum.at(out, dst, msg)
    return np.where(np.isneginf(out), 0.0, out).astype(np.float32)


def _run_spmd(nc, in_maps):
    # the shared axon device occasionally reports a transient
    # NRT_EXEC_UNIT_UNRECOVERABLE on a cold first launch; retry once
    import time
    from concourse.bass_utils import run_bass_kernel_spmd

    try:
        return run_bass_kernel_spmd(nc, in_maps, core_ids=list(range(N_CORES)))
    except Exception:
        time.sleep(10.0)
        return run_bass_kernel_spmd(nc, in_maps, core_ids=list(range(N_CORES)))


def kernel(x, edge_index, edge_attr, W, b):
    bf16 = _bf16()
    x = np.ascontiguousarray(x, dtype=np.float32)
    edge_index = np.ascontiguousarray(edge_index, dtype=np.int32)
    W = np.ascontiguousarray(W, dtype=np.float32)
    b = np.ascontiguousarray(b, dtype=np.float32)

    expected_dst = np.repeat(np.arange(N_NODES, dtype=np.int32), DEG)
    if (
        x.shape != (N_NODES, C)
        or edge_index.shape != (2, N_NODES * DEG)
        or not np.array_equal(edge_index[1], expected_dst)
    ):
        return _numpy_fallback(x, edge_index, W, b)

    src = edge_index[0]
    sig = hash(src.tobytes())
    if _cache.get("layout_sig") != sig:
        _cache["layout"] = _make_layout(src)
        _cache["layout_sig"] = sig
        _cache.pop("gather", None)
    orders, chunks, totcols, idx_all = _cache["layout"]

    if "dense" not in _cache:
        _cache["dense"] = _build_dense()
    if "gather" not in _cache:
        _cache["gather"] = _build_gather(chunks, totcols)

    # ---- Launch 1: dense phase, g = x @ W2.T (transposed layout) ----
    W1, W2 = W[:, :C], W[:, C:]
    wt = np.zeros((KAUG, C), dtype=bf16)
    wt[:C, :] = W2.T.astype(bf16)
    in1 = []
    xts = []
    for c in range(N_CORES):
        xw = np.zeros((KAUG, DN), dtype=bf16)
        xw[:C, :NSH] = x[c * NSH : (c + 1) * NSH].T.astype(bf16)
        xw[C, :] = 1.0
        xts.append(xw)
        in1.append({"xw": xw, "wt": wt})
    r1 = _run_spmd(_cache["dense"], in1)

    g_rows = [
        np.ascontiguousarray(np.asarray(r1.results[c]["ga"])[:, :NSH].T)
        for c in range(N_CORES)
    ]
    g_full = np.concatenate(g_rows, axis=0)  # [N, C] bf16
    gpair = np.concatenate(
        [
            g_full.reshape(NPAIR - 1, 2 * C),
            np.full((1, 2 * C), SENT, dtype=g_full.dtype),
        ],
        axis=0,
    )
    gpair = np.ascontiguousarray(gpair)

    # ---- Launch 2: parity-split gathers + grouped segment max + A ----
    wa = np.zeros((KAUG, C), dtype=bf16)
    wa[:C, :] = (W1 - W2).astype(bf16).T
    wa[C, :] = b.astype(bf16)
    in2 = []
    for c in range(N_CORES):
        xw2 = np.zeros((KAUG, NPAD), dtype=bf16)
        xw2[:, :NSH] = xts[c][:, :NSH][:, orders[c]]
        xw2[C, :] = 1.0
        in2.append({"gpair": gpair, "idxd": idx_all[c], "xwd": xw2, "wad": wa})
    r2 = _run_spmd(_cache["gather"], in2)

    out = np.empty((N_NODES, C), dtype=np.float32)
    for c in range(N_CORES):
        osh = np.asarray(r2.results[c]["oshd"])
        res = osh.reshape(P, NT, C).transpose(1, 0, 2).reshape(NPAD, C)[:NSH]
        out[c * NSH + orders[c]] = res.astype(np.float32)
    _cache["last_results"] = (r1, r2)
    return out


# revision 23
# speedup vs baseline: 1.0289x; 1.0062x over previous
"""EdgeConv (PyG, aggr='max') Trainium2 kernel, 8-core SPMD.

Math: out_i = max_{e: dst(e)=i} relu(x_i @ W1.T + (x_src(e) - x_i) @ W2.T + b)
with W = [W1 | W2].  Rewriting:
    msg_e = relu(A_i + g_src(e)),  A = x @ (W1-W2).T + b,  g = x @ W2.T
Since A_i is constant within segment i and relu is monotone:
    out_i = relu(A_i + max_e g_src(e))
The reference's dst is repeat(arange(N), DEG), so segments are 16 consecutive
edges and the segment-max is a fixed-group reduce after routing each edge's
g row to its slot.

Two SPMD launches on 8 cores:

L1 (dense, node-parallel): host supplies the 6250-node shard pre-transposed,
   xT_aug [65, 6400] bf16, and wt [65, 64] = W2.T (+ zero ones-row).  One
   matmul per 512 nodes emits gT [64, 512] -> bulk store of gT [64, 6400].
   No PE transposes, all DMA transfers are bulk.

L2 (gather + segment max, edge-parallel): the g table is kept as 256B-stride
   pair rows gpair[r] = [g_{2r} | g_{2r+1}] (row 25000 = -3e38 sentinel), but
   each edge's descriptor transfers only the 128B half it needs: dma_gather's
   HBM address math is idx*stride_bytes_256*256 with the transfer size set
   independently by elem_size, so gathers with elem_size=64 bf16 from base
   +0B / +128B views fetch exactly g[src] for even/odd src with idx = src>>1
   (fits int16).  elem_size_bytes%256==0 is only a transpose-mode ucode
   restriction, so the instruction is emitted directly (the bass helper
   over-asserts).  Each node's 16 edges split unevenly between the two
   parity gathers, so the host sorts nodes by even-source count, tiles the
   sorted order into 128-node tiles, and gives each run of tiles uniform
   slot counts (K_even, K_odd) padded with sentinel indices; the segment max
   is then an in-place log2-halving elementwise max over the slot axis.
   A (+ bias, in grouped node order) is recomputed inside this launch on the
   otherwise-idle PE — per 128-node tile, matmul(lhsT=x_tile_aug, rhs=V1_aug)
   lands A directly node-major in PSUM, copied to SBUF on the idle ACT
   engine — then combine = relu(max(even,odd) + A).  All index/permutation
   prep is host-side; the host un-permutes the output rows at the end.
"""

import numpy as np

N_NODES = 50000
DEG = 16
C = 64
N_CORES = 8
NSH = N_NODES // N_CORES  # 6250 nodes per core
P = 128
NT = -(-NSH // P)  # 49 tiles of 128 sorted nodes
NPAD = NT * P  # 6272
NPAIR = N_NODES // 2 + 1  # pair rows + sentinel row
SENT_ROW = N_NODES // 2  # 25000
SENT = -3.0e38
KAUG = C + 1  # x channels + ones row
DN = 6400  # dense-phase padded node count (50 tiles)
MM_CH = 512  # matmul chunk (one PSUM bank)
MAXSL = 64  # max per-partition slots per gather instruction
MAXB = 16  # bucket budget after merging

_cache = {}


def _bf16():
    import ml_dtypes

    return ml_dtypes.bfloat16


def _raw_gather(nc, out_ap, in_ap, idxs_ap, num_idxs):
    """dma_gather with elem_size=64 bf16 (128B payload) on a 256B-stride
    table: bass.dma_gather asserts elem_size_bytes%256==0, but the ucode
    only needs that for transpose mode; emit the instruction directly."""
    import concourse.mybir as mybir

    g = nc.gpsimd
    elem_step = in_ap.ap[0][0]
    stride_bytes = elem_step * mybir.dt.size(in_ap.dtype)
    assert stride_bytes % 256 == 0 and stride_bytes // 256 < 256
    elem_size = in_ap.ap[-1][1]
    assert out_ap.ap[-1][1] == elem_size
    assert out_ap.ap[0][1] * out_ap.ap[1][1] == num_idxs and num_idxs % 128 == 0
    return g.add_instruction(
        mybir.InstDMAGatherAnt(
            name=g.bass.get_next_instruction_name(),
            ins=[
                *g.lower_ap_dma(in_ap, for_custom_bir_dma=True),
                g.lower_ap(idxs_ap),
                g.lower_val_access(g.to_reg(num_idxs)),
            ],
            outs=[g.lower_ap(out_ap)],
            transpose=False,
            num_idxs=num_idxs,
            elem_size=elem_size,
            stride_bytes_256=stride_bytes // 256,
            gen_mode=0,
            single_packet=False,
            queue_num=0,
            sbuf_tokens_per_rank=0,
            sbuf_free_dim_per_rank=0,
            sbuf_free_dim_pad_per_rank=0,
            sbuf_byte_offset=0,
        )
    )


def _build_dense():
    import concourse.bacc as bacc
    import concourse.mybir as mybir
    from concourse.tile import TileContext

    nc = bacc.Bacc("TRN2", target_bir_lowering=False, debug=False)
    bf16 = mybir.dt.bfloat16
    xw = nc.dram_tensor("xw", [KAUG, DN], bf16, kind="ExternalInput")
    wt = nc.dram_tensor("wt", [KAUG, C], bf16, kind="ExternalInput")
    ga = nc.dram_tensor("ga", [C, DN], bf16, kind="ExternalOutput")

    with TileContext(nc) as tc:
        with (
            tc.tile_pool(name="const", bufs=1) as cpool,
            tc.tile_pool(name="sbuf", bufs=1) as pool,
            tc.tile_pool(name="psum", bufs=4, space="PSUM") as psum,
        ):
            wt_sb = cpool.tile([KAUG, C], bf16)
            # tiny weight load rides the idle Pool DMA path, keeping the
            # exclusive HWDGE queue free for the xw pieces
            nc.gpsimd.dma_start(out=wt_sb[:], in_=wt[:])
            xw_sb = pool.tile([KAUG, DN], bf16)
            # small first piece so the first matmul starts early; the output
            # is written back in pieces to overlap the store with compute
            # (copies stay on DVE/ACT — GPSIMD has no PSUM access path)
            in_offs = [(0, 1024)] + [
                (o, min(1344, DN - o)) for o in range(1024, DN, 1344)
            ]
            for o, w in in_offs:
                nc.sync.dma_start(out=xw_sb[:, o : o + w], in_=xw[:, o : o + w])
            ga_sb = pool.tile([C, DN], bf16)
            nouts, wstep = 0, DN // 4
            for j, off in enumerate(range(0, DN, MM_CH)):
                w = min(MM_CH, DN - off)
                h = psum.tile([C, MM_CH], mybir.dt.float32, tag="h")
                nc.tensor.matmul(
                    out=h[:, 0:w],
                    lhsT=wt_sb[:],
                    rhs=xw_sb[:, off : off + w],
                    start=True,
                    stop=True,
                )
                if j % 2 == 0:
                    nc.vector.tensor_copy(out=ga_sb[:, off : off + w], in_=h[:, 0:w])
                else:
                    nc.scalar.copy(out=ga_sb[:, off : off + w], in_=h[:, 0:w])
                while (nouts + 1) * wstep <= off + w:
                    o = nouts * wstep
                    eng = nc.gpsimd if nouts == 0 else nc.sync
                    eng.dma_start(out=ga[:, o : o + wstep], in_=ga_sb[:, o : o + wstep])
                    nouts += 1
    nc.compile()
    return nc


def _halve_max(nc, v, k):
    """In-place log2 max-tree over the slot axis of v [p, t, k, c]; returns
    the [p, t, c] partial-AP at slot 0."""
    while k > 1:
        h = k // 2
        nc.vector.tensor_max(
            out=v[:, :, 0:h, :], in0=v[:, :, 0:h, :], in1=v[:, :, h : 2 * h, :]
        )
        if k % 2:
            nc.vector.tensor_max(
                out=v[:, :, 0:1, :],
                in0=v[:, :, 0:1, :],
                in1=v[:, :, 2 * h : 2 * h + 1, :],
            )
        k = h
    return v[:, :, 0, :]


def _build_gather(chunks, totcols):
    """chunks: list of (t0, T, Ke, Ko, coff_e, coff_o) with uniform slot
    counts per 128-node tile; coff_* are column offsets into the packed idx
    tensor (SBUF layout [32, totcols] int16).  Also recomputes A (grouped
    node order) on the otherwise-idle PE from xwd/wad."""
    import concourse.bacc as bacc
    import concourse.mybir as mybir
    from concourse.tile import TileContext

    nc = bacc.Bacc(
        "TRN2", target_bir_lowering=False, debug=False,
        dynamic_dma_scratch_size=65536,
    )
    bf16 = mybir.dt.bfloat16
    i16 = mybir.dt.int16
    gpair = nc.dram_tensor("gpair", [NPAIR, 2 * C], bf16, kind="ExternalInput")
    idxd = nc.dram_tensor("idxd", [32, totcols], i16, kind="ExternalInput")
    xwd = nc.dram_tensor("xwd", [KAUG, NPAD], bf16, kind="ExternalInput")
    wad = nc.dram_tensor("wad", [KAUG, C], bf16, kind="ExternalInput")
    oshd = nc.dram_tensor("oshd", [P, NT * C], bf16, kind="ExternalOutput")

    # emission order: interleave biggest and smallest chunks so the Pool
    # desc-gen of small (gen-bound) chunks hides behind big transfers
    nsl = lambda ch: ch[1] * (ch[2] + ch[3])
    by_size = sorted(range(len(chunks)), key=lambda i: -nsl(chunks[i]))
    seq = []
    lo_i, hi_i = 0, len(by_size) - 1
    while lo_i <= hi_i:
        seq.append(by_size[lo_i])
        lo_i += 1
        if lo_i <= hi_i:
            seq.append(by_size[hi_i])
            hi_i -= 1
    seq = [chunks[i] for i in seq]

    with TileContext(nc) as tc:
        with (
            tc.tile_pool(name="const", bufs=1) as cpool,
            tc.tile_pool(name="gat", bufs=4) as gpool,
            tc.tile_pool(name="psum", bufs=4, space="PSUM") as psum,
        ):
            idx_sb = cpool.tile([32, totcols], i16)
            # first idx piece covers the first emitted chunk's columns so its
            # desc-gen can start while the bulk loads behind it
            t0, T, ke, ko, ce, co = seq[0]
            lo = min(ce, co)
            hi = max(ce + T * P * ke // 16, co + T * P * ko // 16)
            nc.sync.dma_start(out=idx_sb[:, lo:hi], in_=idxd[:, lo:hi])
            if lo > 0:
                nc.sync.dma_start(out=idx_sb[:, 0:lo], in_=idxd[:, 0:lo])
            if hi < totcols:
                nc.sync.dma_start(out=idx_sb[:, hi:totcols], in_=idxd[:, hi:totcols])
            wa_sb = cpool.tile([KAUG, C], bf16)
            nc.sync.dma_start(out=wa_sb[:], in_=wad[:])
            xw_sb = cpool.tile([KAUG, NPAD], bf16)
            nc.sync.dma_start(out=xw_sb[:], in_=xwd[:])
            ash_sb = cpool.tile([P, NT, C], bf16)
            m_sb = cpool.tile([P, NT, C], bf16)
            o_sb = cpool.tile([P, NT, C], bf16)
            done = [False] * NT
            written = [False] * NT

            def flush(minrun, eng=None):
                # ship every completed-but-unwritten run of >= minrun tiles
                t = 0
                while t < NT:
                    if done[t] and not written[t]:
                        u = t
                        while u < NT and done[u] and not written[u]:
                            u += 1
                        if u - t >= minrun:
                            (eng or nc.sync).dma_start(
                                out=oshd[:, t * C : u * C].rearrange(
                                    "p (t c) -> p t c", c=C
                                ),
                                in_=o_sb[:, t:u, :],
                            )
                            for q in range(t, u):
                                written[q] = True
                        t = u
                    else:
                        t += 1

            for ci, (t0, T, ke, ko, coff_e, coff_o) in enumerate(seq):
                # A tiles for this chunk: matmul(lhsT=x_tile_aug, rhs=V1_aug)
                # lands [128 nodes, 64] node-major; copy on the idle ACT engine
                for t in range(t0, t0 + T):
                    ha = psum.tile([P, C], mybir.dt.float32, tag="ha")
                    nc.tensor.matmul(
                        out=ha[:],
                        lhsT=xw_sb[:, t * P : (t + 1) * P],
                        rhs=wa_sb[:],
                        start=True,
                        stop=True,
                    )
                    nc.scalar.copy(out=ash_sb[:, t, :], in_=ha[:])
                parts = []
                plist = [
                    (ke, coff_e, "ge", gpair[:, 0:C]),
                    (ko, coff_o, "go", gpair[:, C : 2 * C]),
                ]
                if ci == 0 and ko < ke:
                    # the first chunk gates pipeline start on its first
                    # desc-gen — emit the smaller parity gather first
                    plist = plist[::-1]
                for k, coff, tag, base in plist:
                    if k == 0:
                        continue
                    gt = gpool.tile([P, MAXSL, C], bf16, tag=tag)
                    n = T * P * k
                    _raw_gather(
                        nc, gt[:, 0 : T * k, :], base,
                        idx_sb[:, coff : coff + n // 16], n,
                    )
                    v = gt[:, 0 : T * k, :].rearrange("p (t k) c -> p t k c", k=k)
                    parts.append(_halve_max(nc, v, k))
                tgt = m_sb[:, t0 : t0 + T, :]
                if len(parts) == 2:
                    nc.vector.tensor_max(out=tgt, in0=parts[0], in1=parts[1])
                    nc.vector.tensor_add(
                        out=tgt, in0=tgt, in1=ash_sb[:, t0 : t0 + T, :]
                    )
                else:
                    nc.vector.tensor_add(
                        out=tgt, in0=parts[0], in1=ash_sb[:, t0 : t0 + T, :]
                    )
                if ci == len(seq) - 1:
                    # keep the final chunk's chain on DVE — skips the
                    # cross-engine hop on the tail critical path
                    nc.vector.tensor_scalar_max(
                        out=o_sb[:, t0 : t0 + T, :], in0=tgt, scalar1=0.0
                    )
                else:
                    nc.scalar.activation(
                        out=o_sb[:, t0 : t0 + T, :],
                        in_=tgt,
                        func=mybir.ActivationFunctionType.Relu,
                    )
                for t in range(t0, t0 + T):
                    done[t] = True
                pref = NT if all(done) else done.index(False)
                if (
                    sum(done) - sum(written) >= 14
                    and ci < len(seq) - 1
                    and pref - sum(written) >= 12
                ):
                    flush(12)
            flush(1)
    nc.compile()
    return nc


def _make_layout(src):
    """Host-side grouping: sort each core's nodes by even-source count,
    tile the sorted order into 128-node tiles, assign uniform (Ke, Ko)
    slot counts per run of tiles (shared by all 8 cores), and emit the
    chunked instruction table plus per-core packed idx arrays."""
    orders, es = [], []
    for c in range(N_CORES):
        s = src[c * NSH * DEG : (c + 1) * NSH * DEG].reshape(NSH, DEG)
        e = ((s & 1) == 0).sum(1)
        order = np.argsort(-e, kind="stable")
        orders.append(order)
        es.append(e[order])
    es = np.stack(es)  # [8, NSH] descending per row

    ke_t = [int(es[:, t * P].max()) for t in range(NT)]
    ko_t = [int(DEG - es[:, min(t * P + P, NSH) - 1].min()) for t in range(NT)]

    buckets = []  # [t0, t1, Ke, Ko]
    for t in range(NT):
        if buckets and buckets[-1][2] == ke_t[t] and buckets[-1][3] == ko_t[t]:
            buckets[-1][1] = t + 1
        else:
            buckets.append([t, t + 1, ke_t[t], ko_t[t]])

    def cost(b):
        return (b[1] - b[0]) * (b[2] + b[3])

    while len(buckets) > MAXB:
        best, bi = None, None
        for i in range(len(buckets) - 1):
            a, b = buckets[i], buckets[i + 1]
            add = cost([a[0], b[1], max(a[2], b[2]), max(a[3], b[3])]) - cost(a) - cost(b)
            if best is None or add < best:
                best, bi = add, i
        a, b = buckets[bi], buckets[bi + 1]
        buckets[bi : bi + 2] = [[a[0], b[1], max(a[2], b[2]), max(a[3], b[3])]]

    # split buckets so no gather instruction exceeds MAXSL per-partition slots
    chunks = []  # (t0, T, Ke, Ko, coff_e, coff_o)
    coff = 0
    for t0, t1, ke, ko in buckets:
        step = max(1, MAXSL // max(ke, ko, 1))
        t = t0
        while t < t1:
            T = min(step, t1 - t)
            ce = coff
            coff += T * P * ke // 16
            co = coff
            coff += T * P * ko // 16
            chunks.append((t, T, ke, ko, ce, co))
            t += T
    totcols = coff

    def core_idx(c):
        s = src[c * NSH * DEG : (c + 1) * NSH * DEG].reshape(NSH, DEG)
        ss = s[orders[c]]
        par = ss & 1
        e = (par == 0).sum(1)
        key = np.argsort(par, axis=1, kind="stable")  # evens first
        pr = np.take_along_axis(ss >> 1, key, axis=1).astype(np.int16)
        cols = np.arange(DEG)[None, :]
        ev = np.where(cols < e[:, None], pr, np.int16(SENT_ROW))
        oc = e[:, None] + cols
        od = np.where(
            oc < DEG,
            np.take_along_axis(pr, np.minimum(oc, DEG - 1), axis=1),
            np.int16(SENT_ROW),
        )
        ev_pad = np.full((NPAD, DEG), SENT_ROW, np.int16)
        od_pad = np.full((NPAD, DEG), SENT_ROW, np.int16)
        ev_pad[:NSH] = ev
        od_pad[:NSH] = od
        pieces = []
        for t0, T, ke, ko, _, _ in chunks:
            for arr, k in ((ev_pad, ke), (od_pad, ko)):
                if k == 0:
                    continue
                blk = (
                    arr[t0 * P : (t0 + T) * P, 0:k]
                    .reshape(T, P, k)
                    .transpose(0, 2, 1)
                    .reshape(-1)
                )
                w = blk.reshape(-1, 16).T  # [16, n/16]
                pieces.append(np.tile(w, (2, 1)))  # [32, n/16]
        return np.ascontiguousarray(np.concatenate(pieces, axis=1))

    idx_all = [core_idx(c) for c in range(N_CORES)]
    assert idx_all[0].shape == (32, totcols), (idx_all[0].shape, totcols)
    return orders, chunks, totcols, idx_all


def _numpy_fallback(x, edge_index, W, b):
    src, dst = edge_index[0], edge_index[1]
    V1 = W[:, :C] - W[:, C:]
    V2 = W[:, C:]
    A = x @ V1.T + b
    g = x @ V2.T
    out = np.full((x.shape[0], C), -np.inf, dtype=np.float32)
    msg = np.maximum(A[dst] + g[src], 0.0)
    np.maximum.at(out, dst, msg)
    return np.where(np.isneginf(out), 0.0, out).astype(np.float32)


def _run_spmd(nc, in_maps):
    # the shared axon device occasionally reports a transient
    # NRT_EXEC_UNIT_UNRECOVERABLE on a cold first launch; retry once
    import time
    from concourse.bass_utils import run_bass_kernel_spmd

    try:
        return run_bass_kernel_spmd(nc, in_maps, core_ids=list(range(N_CORES)))
    except Exception:
        time.sleep(10.0)
        return run_bass_kernel_spmd(nc, in_maps, core_ids=list(range(N_CORES)))


def kernel(x, edge_index, edge_attr, W, b):
    bf16 = _bf16()
    x = np.ascontiguousarray(x, dtype=np.float32)
    edge_index = np.ascontiguousarray(edge_index, dtype=np.int32)
    W = np.ascontiguousarray(W, dtype=np.float32)
    b = np.ascontiguousarray(b, dtype=np.float32)

    expected_dst = np.repeat(np.arange(N_NODES, dtype=np.int32), DEG)
    if (
        x.shape != (N_NODES, C)
        or edge_index.shape != (2, N_NODES * DEG)
        or not np.array_equal(edge_index[1], expected_dst)
    ):
        return _numpy_fallback(x, edge_index, W, b)

    src = edge_index[0]
    sig = hash(src.tobytes())
    if _cache.get("layout_sig") != sig:
        _cache["layout"] = _make_layout(src)
        _cache["layout_sig"] = sig
        _cache.pop("gather", None)
    orders, chunks, totcols, idx_all = _cache["layout"]

    if "dense" not in _cache:
        _cache["dense"] = _build_dense()
    if "gather" not in _cache:
        _cache["gather"] = _build_gather(chunks, totcols)

    # ---- Launch 1: dense phase, g = x @ W2.T (transposed layout) ----
    W1, W2 = W[:, :C], W[:, C:]
    wt = np.zeros((KAUG, C), dtype=bf16)
    wt[:C, :] = W2.T.astype(bf16)
    in1 = []
    xts = []
    for c in range(N_CORES):
        xw = np.zeros((KAUG, DN), dtype=bf16)
        xw[:C, :NSH] = x[c * NSH : (c + 1) * NSH].T.astype(bf16)
        xw[C, :] = 1.0
        xts.append(xw)
        in1.append({"xw": xw, "wt": wt})
    r1 = _run_spmd(_cache["dense"], in1)

    g_rows = [
        np.ascontiguousarray(np.asarray(r1.results[c]["ga"])[:, :NSH].T)
        for c in range(N_CORES)
    ]
    g_full = np.concatenate(g_rows, axis=0)  # [N, C] bf16
    gpair = np.concatenate(
        [
            g_full.reshape(NPAIR - 1, 2 * C),
            np.full((1, 2 * C), SENT, dtype=g_full.dtype),
        ],
        axis=0,
    )
    gpair = np.ascontiguousarray(gpair)

    # ---- Launch 2: parity-split gathers + grouped segment max + A ----
    wa = np.zeros((KAUG, C), dtype=bf16)
    wa[:C, :] = (W1 - W2).astype(bf16).T
    wa[C, :] = b.astype(bf16)
    in2 = []
    for c in range(N_CORES):
        xw2 = np.zeros((KAUG, NPAD), dtype=bf16)
        xw2[:, :NSH] = xts[c][:, :NSH][:, orders[c]]
        xw2[C, :] = 1.0
        in2.append({"gpair": gpair, "idxd": idx_all[c], "xwd": xw2, "wad": wa})
    r2 = _run_spmd(_cache["gather"], in2)

    out = np.empty((N_NODES, C), dtype=np.float32)
    for c in range(N_CORES):
        osh = np.asarray(r2.results[c]["oshd"])
        res = osh.reshape(P, NT, C).transpose(1, 0, 2).reshape(NPAD, C)[:NSH]
        out[c * NSH + orders[c]] = res.astype(np.float32)
    _cache["last_results"] = (r1, r2)
    return out


# revision 26
# speedup vs baseline: 1.0316x; 1.0027x over previous
"""EdgeConv (PyG, aggr='max') Trainium2 kernel, 8-core SPMD.

Math: out_i = max_{e: dst(e)=i} relu(x_i @ W1.T + (x_src(e) - x_i) @ W2.T + b)
with W = [W1 | W2].  Rewriting:
    msg_e = relu(A_i + g_src(e)),  A = x @ (W1-W2).T + b,  g = x @ W2.T
Since A_i is constant within segment i and relu is monotone:
    out_i = relu(A_i + max_e g_src(e))
The reference's dst is repeat(arange(N), DEG), so segments are 16 consecutive
edges and the segment-max is a fixed-group reduce after routing each edge's
g row to its slot.

Two SPMD launches on 8 cores:

L1 (dense, node-parallel): host supplies the 6250-node shard pre-transposed,
   xT_aug [65, 6400] bf16, and wt [65, 64] = W2.T (+ zero ones-row).  One
   matmul per 512 nodes emits gT [64, 512] -> bulk store of gT [64, 6400].
   No PE transposes, all DMA transfers are bulk.

L2 (gather + segment max, edge-parallel): the g table is kept as 256B-stride
   pair rows gpair[r] = [g_{2r} | g_{2r+1}] (row 25000 = -3e38 sentinel), but
   each edge's descriptor transfers only the 128B half it needs: dma_gather's
   HBM address math is idx*stride_bytes_256*256 with the transfer size set
   independently by elem_size, so gathers with elem_size=64 bf16 from base
   +0B / +128B views fetch exactly g[src] for even/odd src with idx = src>>1
   (fits int16).  elem_size_bytes%256==0 is only a transpose-mode ucode
   restriction, so the instruction is emitted directly (the bass helper
   over-asserts).  Each node's 16 edges split unevenly between the two
   parity gathers, so the host sorts nodes by even-source count, tiles the
   sorted order into 128-node tiles, and gives each run of tiles uniform
   slot counts (K_even, K_odd) padded with sentinel indices; the segment max
   is then an in-place log2-halving elementwise max over the slot axis.
   A (+ bias, in grouped node order) is recomputed inside this launch on the
   otherwise-idle PE — per 128-node tile, matmul(lhsT=x_tile_aug, rhs=V1_aug)
   lands A directly node-major in PSUM, copied to SBUF on the idle ACT
   engine — then combine = relu(max(even,odd) + A).  All index/permutation
   prep is host-side; the host un-permutes the output rows at the end.
"""

import numpy as np

N_NODES = 50000
DEG = 16
C = 64
N_CORES = 8
NSH = N_NODES // N_CORES  # 6250 nodes per core
P = 128
NT = -(-NSH // P)  # 49 tiles of 128 sorted nodes
NPAD = NT * P  # 6272
NPAIR = N_NODES // 2 + 1  # pair rows + sentinel row
SENT_ROW = N_NODES // 2  # 25000
SENT = -3.0e38
KAUG = C + 1  # x channels + ones row
DN = 6400  # dense-phase padded node count (50 tiles)
MM_CH = 512  # matmul chunk (one PSUM bank)
MAXSL = 64  # max per-partition slots per gather instruction
MAXB = 16  # bucket budget after merging

_cache = {}


def _bf16():
    import ml_dtypes

    return ml_dtypes.bfloat16


def _raw_gather(nc, out_ap, in_ap, idxs_ap, num_idxs):
    """dma_gather with elem_size=64 bf16 (128B payload) on a 256B-stride
    table: bass.dma_gather asserts elem_size_bytes%256==0, but the ucode
    only needs that for transpose mode; emit the instruction directly."""
    import concourse.mybir as mybir

    g = nc.gpsimd
    elem_step = in_ap.ap[0][0]
    stride_bytes = elem_step * mybir.dt.size(in_ap.dtype)
    assert stride_bytes % 256 == 0 and stride_bytes // 256 < 256
    elem_size = in_ap.ap[-1][1]
    assert out_ap.ap[-1][1] == elem_size
    assert out_ap.ap[0][1] * out_ap.ap[1][1] == num_idxs and num_idxs % 128 == 0
    return g.add_instruction(
        mybir.InstDMAGatherAnt(
            name=g.bass.get_next_instruction_name(),
            ins=[
                *g.lower_ap_dma(in_ap, for_custom_bir_dma=True),
                g.lower_ap(idxs_ap),
                g.lower_val_access(g.to_reg(num_idxs)),
            ],
            outs=[g.lower_ap(out_ap)],
            transpose=False,
            num_idxs=num_idxs,
            elem_size=elem_size,
            stride_bytes_256=stride_bytes // 256,
            gen_mode=0,
            single_packet=False,
            queue_num=0,
            sbuf_tokens_per_rank=0,
            sbuf_free_dim_per_rank=0,
            sbuf_free_dim_pad_per_rank=0,
            sbuf_byte_offset=0,
        )
    )


def _build_dense():
    import concourse.bacc as bacc
    import concourse.mybir as mybir
    from concourse.tile import TileContext

    nc = bacc.Bacc("TRN2", target_bir_lowering=False, debug=False)
    bf16 = mybir.dt.bfloat16
    xw = nc.dram_tensor("xw", [KAUG, DN], bf16, kind="ExternalInput")
    wt = nc.dram_tensor("wt", [KAUG, C], bf16, kind="ExternalInput")
    ga = nc.dram_tensor("ga", [C, DN], bf16, kind="ExternalOutput")

    with TileContext(nc) as tc:
        with (
            tc.tile_pool(name="const", bufs=1) as cpool,
            tc.tile_pool(name="sbuf", bufs=1) as pool,
            tc.tile_pool(name="psum", bufs=4, space="PSUM") as psum,
        ):
            wt_sb = cpool.tile([KAUG, C], bf16)
            # tiny weight load rides the idle Pool DMA path, keeping the
            # exclusive HWDGE queue free for the xw pieces
            nc.gpsimd.dma_start(out=wt_sb[:], in_=wt[:])
            xw_sb = pool.tile([KAUG, DN], bf16)
            # small first piece so the first matmul starts early; the output
            # is written back in pieces to overlap the store with compute
            # (copies stay on DVE/ACT — GPSIMD has no PSUM access path)
            in_offs = [(0, 1024)] + [
                (o, min(1344, DN - o)) for o in range(1024, DN, 1344)
            ]
            for o, w in in_offs:
                nc.sync.dma_start(out=xw_sb[:, o : o + w], in_=xw[:, o : o + w])
            ga_sb = pool.tile([C, DN], bf16)
            nouts, wstep = 0, DN // 4
            for j, off in enumerate(range(0, DN, MM_CH)):
                w = min(MM_CH, DN - off)
                h = psum.tile([C, MM_CH], mybir.dt.float32, tag="h")
                nc.tensor.matmul(
                    out=h[:, 0:w],
                    lhsT=wt_sb[:],
                    rhs=xw_sb[:, off : off + w],
                    start=True,
                    stop=True,
                )
                if j % 2 == 0:
                    nc.vector.tensor_copy(out=ga_sb[:, off : off + w], in_=h[:, 0:w])
                else:
                    nc.scalar.copy(out=ga_sb[:, off : off + w], in_=h[:, 0:w])
                while (nouts + 1) * wstep <= off + w:
                    o = nouts * wstep
                    eng = nc.gpsimd if nouts == 0 else nc.sync
                    eng.dma_start(out=ga[:, o : o + wstep], in_=ga_sb[:, o : o + wstep])
                    nouts += 1
    nc.compile()
    return nc


def _halve_max(nc, v, k):
    """In-place log2 max-tree over the slot axis of v [p, t, k, c]; returns
    the [p, t, c] partial-AP at slot 0."""
    while k > 1:
        h = k // 2
        nc.vector.tensor_max(
            out=v[:, :, 0:h, :], in0=v[:, :, 0:h, :], in1=v[:, :, h : 2 * h, :]
        )
        if k % 2:
            nc.vector.tensor_max(
                out=v[:, :, 0:1, :],
                in0=v[:, :, 0:1, :],
                in1=v[:, :, 2 * h : 2 * h + 1, :],
            )
        k = h
    return v[:, :, 0, :]


def _interleave_order(chunks):
    """Default emission order: biggest and smallest chunks alternating, so
    desc-gen of small (gen-bound) chunks hides behind big transfers."""
    nsl = lambda ch: ch[1] * (ch[2] + ch[3])
    big = sorted(range(len(chunks)), key=lambda i: -nsl(chunks[i]))
    seq = []
    lo, hi = 0, len(big) - 1
    while lo <= hi:
        seq.append(big[lo])
        lo += 1
        if lo <= hi:
            seq.append(big[hi])
            hi -= 1
    return seq


def _tune_gather_order(chunks, totcols, trials=8):
    """Deterministic local search over emission orders, scored with the
    timeline model at build time (host-side only; the chosen program is
    rebuilt once).  Falls back to the interleave on any failure."""
    import random

    from concourse.timeline_sim import TimelineSim

    base = _interleave_order(chunks)
    try:
        best = (TimelineSim(_build_gather(chunks, totcols, base)).simulate(), base)
        rng = random.Random(7)
        n = len(chunks)
        for _ in range(trials):
            o = best[1][:]
            for _ in range(rng.randint(1, 3)):
                i, j = rng.randrange(n), rng.randrange(n)
                o[i], o[j] = o[j], o[i]
            t = TimelineSim(_build_gather(chunks, totcols, o)).simulate()
            if t < best[0]:
                best = (t, o)
        return best[1]
    except Exception:
        return base


def _build_gather(chunks, totcols, order=None):
    """chunks: list of (t0, T, Ke, Ko, coff_e, coff_o) with uniform slot
    counts per 128-node tile; coff_* are column offsets into the packed idx
    tensor (SBUF layout [32, totcols] int16).  Also recomputes A (grouped
    node order) on the otherwise-idle PE from xwd/wad.  `order` overrides
    the emission order (indices into chunks)."""
    import concourse.bacc as bacc
    import concourse.mybir as mybir
    from concourse.tile import TileContext

    nc = bacc.Bacc(
        "TRN2", target_bir_lowering=False, debug=False,
        dynamic_dma_scratch_size=65536,
    )
    bf16 = mybir.dt.bfloat16
    i16 = mybir.dt.int16
    gpair = nc.dram_tensor("gpair", [NPAIR, 2 * C], bf16, kind="ExternalInput")
    idxd = nc.dram_tensor("idxd", [32, totcols], i16, kind="ExternalInput")
    xwd = nc.dram_tensor("xwd", [KAUG, NPAD], bf16, kind="ExternalInput")
    wad = nc.dram_tensor("wad", [KAUG, C], bf16, kind="ExternalInput")
    oshd = nc.dram_tensor("oshd", [P, NT * C], bf16, kind="ExternalOutput")

    seq = [chunks[i] for i in (order if order is not None else _interleave_order(chunks))]

    with TileContext(nc) as tc:
        with (
            tc.tile_pool(name="const", bufs=1) as cpool,
            tc.tile_pool(name="gat", bufs=4) as gpool,
            tc.tile_pool(name="psum", bufs=4, space="PSUM") as psum,
        ):
            idx_sb = cpool.tile([32, totcols], i16)
            # first idx piece covers the first emitted chunk's columns so its
            # desc-gen can start while the bulk loads behind it
            t0, T, ke, ko, ce, co = seq[0]
            lo = min(ce, co)
            hi = max(ce + T * P * ke // 16, co + T * P * ko // 16)
            nc.sync.dma_start(out=idx_sb[:, lo:hi], in_=idxd[:, lo:hi])
            if lo > 0:
                nc.sync.dma_start(out=idx_sb[:, 0:lo], in_=idxd[:, 0:lo])
            if hi < totcols:
                nc.sync.dma_start(out=idx_sb[:, hi:totcols], in_=idxd[:, hi:totcols])
            wa_sb = cpool.tile([KAUG, C], bf16)
            nc.sync.dma_start(out=wa_sb[:], in_=wad[:])
            xw_sb = cpool.tile([KAUG, NPAD], bf16)
            nc.sync.dma_start(out=xw_sb[:], in_=xwd[:])
            ash_sb = cpool.tile([P, NT, C], bf16)
            m_sb = cpool.tile([P, NT, C], bf16)
            o_sb = cpool.tile([P, NT, C], bf16)
            done = [False] * NT
            written = [False] * NT

            def flush(minrun, eng=None):
                # ship every completed-but-unwritten run of >= minrun tiles
                t = 0
                while t < NT:
                    if done[t] and not written[t]:
                        u = t
                        while u < NT and done[u] and not written[u]:
                            u += 1
                        if u - t >= minrun:
                            (eng or nc.sync).dma_start(
                                out=oshd[:, t * C : u * C].rearrange(
                                    "p (t c) -> p t c", c=C
                                ),
                                in_=o_sb[:, t:u, :],
                            )
                            for q in range(t, u):
                                written[q] = True
                        t = u
                    else:
                        t += 1

            for ci, (t0, T, ke, ko, coff_e, coff_o) in enumerate(seq):
                # A tiles for this chunk: matmul(lhsT=x_tile_aug, rhs=V1_aug)
                # lands [128 nodes, 64] node-major; copy on the idle ACT engine
                for t in range(t0, t0 + T):
                    ha = psum.tile([P, C], mybir.dt.float32, tag="ha")
                    nc.tensor.matmul(
                        out=ha[:],
                        lhsT=xw_sb[:, t * P : (t + 1) * P],
                        rhs=wa_sb[:],
                        start=True,
                        stop=True,
                    )
                    nc.scalar.copy(out=ash_sb[:, t, :], in_=ha[:])
                parts = []
                plist = [
                    (ke, coff_e, "ge", gpair[:, 0:C]),
                    (ko, coff_o, "go", gpair[:, C : 2 * C]),
                ]
                if ci == 0 and ko < ke:
                    # the first chunk gates pipeline start on its first
                    # desc-gen — emit the smaller parity gather first
                    plist = plist[::-1]
                for k, coff, tag, base in plist:
                    if k == 0:
                        continue
                    gt = gpool.tile([P, MAXSL, C], bf16, tag=tag)
                    n = T * P * k
                    _raw_gather(
                        nc, gt[:, 0 : T * k, :], base,
                        idx_sb[:, coff : coff + n // 16], n,
                    )
                    v = gt[:, 0 : T * k, :].rearrange("p (t k) c -> p t k c", k=k)
                    parts.append(_halve_max(nc, v, k))
                tgt = m_sb[:, t0 : t0 + T, :]
                if len(parts) == 2:
                    nc.vector.tensor_max(out=tgt, in0=parts[0], in1=parts[1])
                    nc.vector.tensor_add(
                        out=tgt, in0=tgt, in1=ash_sb[:, t0 : t0 + T, :]
                    )
                else:
                    nc.vector.tensor_add(
                        out=tgt, in0=parts[0], in1=ash_sb[:, t0 : t0 + T, :]
                    )
                if ci == len(seq) - 1:
                    # keep the final chunk's chain on DVE — skips the
                    # cross-engine hop on the tail critical path
                    nc.vector.tensor_scalar_max(
                        out=o_sb[:, t0 : t0 + T, :], in0=tgt, scalar1=0.0
                    )
                else:
                    nc.scalar.activation(
                        out=o_sb[:, t0 : t0 + T, :],
                        in_=tgt,
                        func=mybir.ActivationFunctionType.Relu,
                    )
                for t in range(t0, t0 + T):
                    done[t] = True
                pref = NT if all(done) else done.index(False)
                if (
                    sum(done) - sum(written) >= 14
                    and ci < len(seq) - 1
                    and pref - sum(written) >= 12
                ):
                    flush(12)
            flush(1)
    nc.compile()
    return nc


def _make_layout(src):
    """Host-side grouping: sort each core's nodes by even-source count,
    tile the sorted order into 128-node tiles, assign uniform (Ke, Ko)
    slot counts per run of tiles (shared by all 8 cores), and emit the
    chunked instruction table plus per-core packed idx arrays."""
    orders, es = [], []
    for c in range(N_CORES):
        s = src[c * NSH * DEG : (c + 1) * NSH * DEG].reshape(NSH, DEG)
        e = ((s & 1) == 0).sum(1)
        order = np.argsort(-e, kind="stable")
        orders.append(order)
        es.append(e[order])
    es = np.stack(es)  # [8, NSH] descending per row

    ke_t = [int(es[:, t * P].max()) for t in range(NT)]
    ko_t = [int(DEG - es[:, min(t * P + P, NSH) - 1].min()) for t in range(NT)]

    buckets = []  # [t0, t1, Ke, Ko]
    for t in range(NT):
        if buckets and buckets[-1][2] == ke_t[t] and buckets[-1][3] == ko_t[t]:
            buckets[-1][1] = t + 1
        else:
            buckets.append([t, t + 1, ke_t[t], ko_t[t]])

    def cost(b):
        return (b[1] - b[0]) * (b[2] + b[3])

    while len(buckets) > MAXB:
        best, bi = None, None
        for i in range(len(buckets) - 1):
            a, b = buckets[i], buckets[i + 1]
            add = cost([a[0], b[1], max(a[2], b[2]), max(a[3], b[3])]) - cost(a) - cost(b)
            if best is None or add < best:
                best, bi = add, i
        a, b = buckets[bi], buckets[bi + 1]
        buckets[bi : bi + 2] = [[a[0], b[1], max(a[2], b[2]), max(a[3], b[3])]]

    # split buckets so no gather instruction exceeds MAXSL per-partition slots
    chunks = []  # (t0, T, Ke, Ko, coff_e, coff_o)
    coff = 0
    for t0, t1, ke, ko in buckets:
        step = max(1, MAXSL // max(ke, ko, 1))
        t = t0
        while t < t1:
            T = min(step, t1 - t)
            ce = coff
            coff += T * P * ke // 16
            co = coff
            coff += T * P * ko // 16
            chunks.append((t, T, ke, ko, ce, co))
            t += T
    totcols = coff

    def core_idx(c):
        s = src[c * NSH * DEG : (c + 1) * NSH * DEG].reshape(NSH, DEG)
        ss = s[orders[c]]
        par = ss & 1
        e = (par == 0).sum(1)
        key = np.argsort(par, axis=1, kind="stable")  # evens first
        pr = np.take_along_axis(ss >> 1, key, axis=1).astype(np.int16)
        cols = np.arange(DEG)[None, :]
        ev = np.where(cols < e[:, None], pr, np.int16(SENT_ROW))
        oc = e[:, None] + cols
        od = np.where(
            oc < DEG,
            np.take_along_axis(pr, np.minimum(oc, DEG - 1), axis=1),
            np.int16(SENT_ROW),
        )
        ev_pad = np.full((NPAD, DEG), SENT_ROW, np.int16)
        od_pad = np.full((NPAD, DEG), SENT_ROW, np.int16)
        ev_pad[:NSH] = ev
        od_pad[:NSH] = od
        pieces = []
        for t0, T, ke, ko, _, _ in chunks:
            for arr, k in ((ev_pad, ke), (od_pad, ko)):
                if k == 0:
                    continue
                blk = (
                    arr[t0 * P : (t0 + T) * P, 0:k]
                    .reshape(T, P, k)
                    .transpose(0, 2, 1)
                    .reshape(-1)
                )
                w = blk.reshape(-1, 16).T  # [16, n/16]
                pieces.append(np.tile(w, (2, 1)))  # [32, n/16]
        return np.ascontiguousarray(np.concatenate(pieces, axis=1))

    idx_all = [core_idx(c) for c in range(N_CORES)]
    assert idx_all[0].shape == (32, totcols), (idx_all[0].shape, totcols)
    return orders, chunks, totcols, idx_all


def _numpy_fallback(x, edge_index, W, b):
    src, dst = edge_index[0], edge_index[1]
    V1 = W[:, :C] - W[:, C:]
    V2 = W[:, C:]
    A = x @ V1.T + b
    g = x @ V2.T
    out = np.full((x.shape[0], C), -np.inf, dtype=np.float32)
    msg = np.maximum(A[dst] + g[src], 0.0)
    np.maximum.at(out, dst, msg)
    return np.where(np.isneginf(out), 0.0, out).astype(np.float32)


def _run_spmd(nc, in_maps):
    # the shared axon device occasionally reports a transient
    # NRT_EXEC_UNIT_UNRECOVERABLE on a cold first launch; retry once
    import time
    from concourse.bass_utils import run_bass_kernel_spmd

    try:
        return run_bass_kernel_spmd(nc, in_maps, core_ids=list(range(N_CORES)))
    except Exception:
        time.sleep(10.0)
        return run_bass_kernel_spmd(nc, in_maps, core_ids=list(range(N_CORES)))


def kernel(x, edge_index, edge_attr, W, b):
    bf16 = _bf16()
    x = np.ascontiguousarray(x, dtype=np.float32)
    edge_index = np.ascontiguousarray(edge_index, dtype=np.int32)
    W = np.ascontiguousarray(W, dtype=np.float32)
    b = np.ascontiguousarray(b, dtype=np.float32)

    expected_dst = np.repeat(np.arange(N_NODES, dtype=np.int32), DEG)
    if (
        x.shape != (N_NODES, C)
        or edge_index.shape != (2, N_NODES * DEG)
        or not np.array_equal(edge_index[1], expected_dst)
    ):
        return _numpy_fallback(x, edge_index, W, b)

    src = edge_index[0]
    sig = hash(src.tobytes())
    if _cache.get("layout_sig") != sig:
        _cache["layout"] = _make_layout(src)
        _cache["layout_sig"] = sig
        _cache.pop("gather", None)
    orders, chunks, totcols, idx_all = _cache["layout"]

    if "dense" not in _cache:
        _cache["dense"] = _build_dense()
    if "gather" not in _cache:
        order = _tune_gather_order(chunks, totcols)
        _cache["gather"] = _build_gather(chunks, totcols, order)

    # ---- Launch 1: dense phase, g = x @ W2.T (transposed layout) ----
    W1, W2 = W[:, :C], W[:, C:]
    wt = np.zeros((KAUG, C), dtype=bf16)
    wt[:C, :] = W2.T.astype(bf16)
    in1 = []
    xts = []
    for c in range(N_CORES):
        xw = np.zeros((KAUG, DN), dtype=bf16)
        xw[:C, :NSH] = x[c * NSH : (c + 1) * NSH].T.astype(bf16)
        xw[C, :] = 1.0
        xts.append(xw)
        in1.append({"xw": xw, "wt": wt})
    r1 = _run_spmd(_cache["dense"], in1)

    g_rows = [
        np.ascontiguousarray(np.asarray(r1.results[c]["ga"])[:, :NSH].T)
        for c in range(N_CORES)
    ]
    g_full = np.concatenate(g_rows, axis=0)  # [N, C] bf16
    gpair = np.concatenate(
        [
            g_full.reshape(NPAIR - 1, 2 * C),
            np.full((1, 2 * C), SENT, dtype=g_full.dtype),
        ],
        axis=0,
    )
    gpair = np.ascontiguousarray(gpair)

    # ---- Launch 2: parity-split gathers + grouped segment max + A ----
    wa = np.zeros((KAUG, C), dtype=bf16)
    wa[:C, :] = (W1 - W2).astype(bf16).T
    wa[C, :] = b.astype(bf16)
    in2 = []
    for c in range(N_CORES):
        xw2 = np.zeros((KAUG, NPAD), dtype=bf16)
        xw2[:, :NSH] = xts[c][:, :NSH][:, orders[c]]
        xw2[C, :] = 1.0
        in2.append({"gpair": gpair, "idxd": idx_all[c], "xwd": xw2, "wad": wa})
    r2 = _run_spmd(_cache["gather"], in2)

    out = np.empty((N_NODES, C), dtype=np.float32)
    for c in range(N_CORES):
        osh = np.asarray(r2.results[c]["oshd"])
        res = osh.reshape(P, NT, C).transpose(1, 0, 2).reshape(NPAD, C)[:NSH]
        out[c * NSH + orders[c]] = res.astype(np.float32)
    _cache["last_results"] = (r1, r2)
    return out
